# revision 1
# baseline (speedup 1.0000x reference)
"""Trainium2 Bass kernel for nn_CombinedMLPMoEModel (moe_routing).

Strategy (8 NeuronCores, pure data parallel on the batch):
 - Host: shard batch 16384 -> 8 x 2048 tokens, pre-transpose x1/x2/x3 to
   feature-major [Din, tok] so every layer's activation sits with its
   contracted dim on SBUF partitions; replicate weights.
 - On chip, everything stays feature-major: out_fm = W.T @ act_fm with
   lhsT = W exactly as stored [Din, Dout].
 - Precision: the routing decision (top-2 of 8 experts) matches the fp32
   reference only if the MLP chain + router logits carry fp32 accuracy.
   Plain fp32 matmul costs 4 cyc/row on the PE; instead the MLP chain
   uses a 3-term fp32r (tf32-like, 1 cyc/row) split:
       W @ x ~= Whi@xhi + Whi@xlo + Wlo@xhi   (error ~1e-7, 3 cyc/row)
   with Whi/Wlo pre-split on the host and xhi/xlo split on chip.
   Expert matmuls + the folded output layer run in bf16 (routing indices
   stay exact; output rel-err ~3e-3).  LayerNorm statistics use single
   fp32r (their error is a per-token scale/shift, routing-order safe).
 - MoE: dense compute of all 8 experts per 512-token megatile in
   token-major PSUM [128 tok, 512], combined with per-token top-2
   softmax weights via scalar_tensor_tensor, then transposed back to
   feature-major on the PE.
 - The tail (concat(o) @ Wf -> bn -> @ Wr) is linear, so it folds on the
   host into one vector: out = concat(o) @ (Wf @ (scf * Wr)) + c0.
"""

import numpy as np
import ml_dtypes
from contextlib import ExitStack

import concourse.bass as bass
from concourse import bacc
import concourse.mybir as mybir
import concourse.tile as tile
from concourse.bass_utils import run_bass_kernel_spmd

F32 = mybir.dt.float32
F32R = mybir.dt.float32r
BF16 = mybir.dt.bfloat16
AF = mybir.ActivationFunctionType
ALU = mybir.AluOpType
AX = mybir.AxisListType

N_CORES = 8
B = 16384
DIN = 1024
D = 512
H = 1024
D3 = 3 * D          # 1536
E = 8
TOK_CORE = B // N_CORES   # 2048
MT = 512                  # megatile tokens
EPS = 1e-5
NEG_BIG = -1.0e30

_PROGRAM_CACHE = {}


def _chunks(n):
    return n // 128


def build_program(n_tok=TOK_CORE, mt=MT):
    """Build the per-core Bass program (SPMD across the 8 cores)."""
    nc = bacc.Bacc(None, target_bir_lowering=False)
    T = n_tok // mt
    CS = mt
    NC_TOK = _chunks(mt)

    # ---------------- DRAM I/O ----------------
    xs = [nc.dram_tensor(f"x{i+1}t", [DIN, n_tok], F32, kind="ExternalInput")
          for i in range(3)]
    Wp = [nc.dram_tensor(f"Wp{i+1}", [DIN, D], F32, kind="ExternalInput")
          for i in range(3)]
    W1 = nc.dram_tensor("W1", [D3, H], F32, kind="ExternalInput")
    W2 = nc.dram_tensor("W2", [H, H], F32, kind="ExternalInput")
    W3 = nc.dram_tensor("W3", [H, D3], F32, kind="ExternalInput")
    # packed per-feature vectors, [128, chunks] layout
    bp = [nc.dram_tensor(f"bp{i+1}", [128, _chunks(D)], F32, kind="ExternalInput")
          for i in range(3)]
    sc1 = nc.dram_tensor("sc1", [128, _chunks(H)], F32, kind="ExternalInput")
    bi1 = nc.dram_tensor("bi1", [128, _chunks(H)], F32, kind="ExternalInput")
    sc2 = nc.dram_tensor("sc2", [128, _chunks(H)], F32, kind="ExternalInput")
    bi2 = nc.dram_tensor("bi2", [128, _chunks(H)], F32, kind="ExternalInput")
    b3v = nc.dram_tensor("b3v", [128, _chunks(D3)], F32, kind="ExternalInput")
    lngv = nc.dram_tensor("lngv", [128, _chunks(D3)], F32, kind="ExternalInput")
    lnbv = nc.dram_tensor("lnbv", [128, _chunks(D3)], F32, kind="ExternalInput")
    Wg_d = nc.dram_tensor("Wg_r", [128, _chunks(D), E], F32, kind="ExternalInput")
    Wfr_d = nc.dram_tensor("Wfr_r", [128, _chunks(D3), 1], BF16, kind="ExternalInput")
    We_d = nc.dram_tensor("We_r", [128, E, _chunks(D), D], BF16, kind="ExternalInput")
    bexp_d = nc.dram_tensor("bexp_bf", [E, D], BF16, kind="ExternalInput")
    bg_d = nc.dram_tensor("bg_v", [1, E], F32, kind="ExternalInput")
    c0_d = nc.dram_tensor("c0_v", [1, 1], F32, kind="ExternalInput")
    ones_d = nc.dram_tensor("ones_col", [128, 1], F32, kind="ExternalInput")
    onesr_d = nc.dram_tensor("ones_row", [1, 128], F32, kind="ExternalInput")
    ident_d = nc.dram_tensor("ident", [128, 128], F32, kind="ExternalInput")
    out_d = nc.dram_tensor("out", [1, n_tok], F32, kind="ExternalOutput")

    with tile.TileContext(nc) as tc, ExitStack() as ctx:
        cp = ctx.enter_context(tc.tile_pool(name="consts", bufs=1))
        sp = ctx.enter_context(tc.tile_pool(name="work", bufs=1))
        ps = ctx.enter_context(tc.tile_pool(name="psum", bufs=8, space="PSUM"))

        def pt(shape, dtype, tag, bufs=None):
            return sp.tile(shape, dtype, tag=tag, bufs=bufs, name=tag)

        def mmtile(name="p"):
            return ps.tile([128, 512], F32, tag="mm", name=name)

        # ---------------- resident constants ----------------
        We_sb = cp.tile([128, E, _chunks(D), D], BF16, name="We_sb")
        Wg_sb = cp.tile([128, _chunks(D), E], F32, name="Wg_sb")
        nc.sync.dma_start(out=Wg_sb, in_=Wg_d[:, :, :])
        Wfr_sb = cp.tile([128, _chunks(D3), 1], BF16, name="Wfr_sb")
        nc.sync.dma_start(out=Wfr_sb, in_=Wfr_d[:, :, :])
        bexp_sb = cp.tile([E, D], BF16, name="bexp_sb")
        nc.sync.dma_start(out=bexp_sb, in_=bexp_d[:, :])
        ident_sb = cp.tile([128, 128], F32, name="ident_sb")
        nc.sync.dma_start(out=ident_sb, in_=ident_d[:, :])
        ones_r = cp.tile([128, 1], F32R, name="ones_r")
        nc.gpsimd.dma_start(out=ones_r, in_=ones_d[:, :])
        onesr_r = cp.tile([1, 128], F32R, name="onesr_r")
        nc.gpsimd.dma_start(out=onesr_r, in_=onesr_d[:, :])
        bg_bc = cp.tile([128, E], F32, name="bg_bc")
        nc.gpsimd.dma_start(
            out=bg_bc,
            in_=bass.AP(tensor=bg_d[:, :].tensor, offset=0, ap=[[0, 128], [1, E]]),
        )
        c0_sb = cp.tile([1, 1], F32, name="c0_sb")
        nc.sync.dma_start(out=c0_sb, in_=c0_d[:, :])

        def ldvec(dram, nch, name):
            t = cp.tile([128, nch], F32, name=name)
            nc.sync.dma_start(out=t, in_=dram[:, :])
            return t

        bp_sb = [ldvec(bp[i], _chunks(D), f"bp{i}_sb") for i in range(3)]
        sc1_sb = ldvec(sc1, _chunks(H), "sc1_sb")
        bi1_sb = ldvec(bi1, _chunks(H), "bi1_sb")
        sc2_sb = ldvec(sc2, _chunks(H), "sc2_sb")
        bi2_sb = ldvec(bi2, _chunks(H), "bi2_sb")
        b3_sb = ldvec(b3v, _chunks(D3), "b3_sb")
        lng_sb = ldvec(lngv, _chunks(D3), "lng_sb")
        lnb_sb = ldvec(lnbv, _chunks(D3), "lnb_sb")

        def split_act(src_ap):
            """tf32 hi/lo split of one [128, CS] fp32 activation chunk."""
            hi = pt([128, CS], F32R, tag="aph", bufs=2)
            nc.scalar.copy(hi, src_ap)
            lo = pt([128, CS], F32R, tag="apl", bufs=2)
            nc.vector.scalar_tensor_tensor(out=lo, in0=src_ap, scalar=-1.0,
                                           in1=hi.bitcast(F32), op0=ALU.bypass,
                                           op1=ALU.subtract)
            return hi, lo

        def load_w_pair(w_dram, k, dgs, dgw):
            """Load fp32 weight chunk [128, dgw] (k-chunk k, dout slice
            [dgs, dgs+dgw)) and split into tf32 hi/lo on chip."""
            wk = pt([128, 1024], F32, tag="wkf", bufs=3)[:, :dgw]
            nc.sync.dma_start(out=wk,
                              in_=w_dram[128 * k:128 * (k + 1), dgs:dgs + dgw])
            wh = pt([128, 1024], F32R, tag="wkh", bufs=2)[:, :dgw]
            nc.scalar.copy(wh, wk)
            wl = pt([128, 1024], F32R, tag="wkl", bufs=2)[:, :dgw]
            nc.vector.scalar_tensor_tensor(out=wl, in0=wk, scalar=-1.0,
                                           in1=wh.bitcast(F32), op0=ALU.bypass,
                                           op1=ALU.subtract)
            return wh, wl

        def mm3(psum, wh, wl, xh, xl, d, start, stop):
            sl = slice(128 * d, 128 * (d + 1))
            nc.tensor.matmul(psum, wh[:, sl], xh, start=start, stop=False)
            nc.tensor.matmul(psum, wh[:, sl], xl, start=False, stop=False)
            nc.tensor.matmul(psum, wl[:, sl], xh, start=False, stop=stop)

        # ---------------- megatile loop ----------------
        for t in range(T):
            ts = slice(t * CS, (t + 1) * CS)

            # ---- stage A: three projections -> comb [128, 12, CS] ----
            comb = pt([128, _chunks(D3), CS], F32, tag="big12", bufs=2)
            for i in range(3):
                psums = [mmtile(f"pp{i}") for _ in range(4)]
                for k in range(_chunks(DIN)):
                    xk = pt([128, CS], F32, tag="wkf", bufs=3)
                    nc.sync.dma_start(out=xk, in_=xs[i][128 * k:128 * (k + 1), ts])
                    xh, xl = split_act(xk)
                    wh, wl = load_w_pair(Wp[i], k, 0, 512)
                    for d in range(4):
                        mm3(psums[d], wh, wl, xh, xl, d,
                            start=(k == 0), stop=(k == _chunks(DIN) - 1))
                for d in range(4):
                    dd = 4 * i + d
                    nc.scalar.activation(comb[:, dd, :], psums[d], AF.Identity,
                                         bias=bp_sb[i][:, d:d + 1], scale=1.0)

            if t == 0:
                nc.sync.dma_start(out=We_sb, in_=We_d[:, :, :, :])

            # ---- W1 -> h1, W2 -> h2: single dout pass, 8 psum banks ----
            def dense_relu_bn(act_in, w_dram, kch, sc_sb, bi_sb):
                hout = pt([128, _chunks(H), CS], F32, tag="h", bufs=2)
                psums = [mmtile("ph") for _ in range(8)]
                for k in range(kch):
                    ah, al = split_act(act_in[:, k, :])
                    wh, wl = load_w_pair(w_dram, k, 0, 1024)
                    for d in range(8):
                        mm3(psums[d], wh, wl, ah, al, d,
                            start=(k == 0), stop=(k == kch - 1))
                for d in range(8):
                    nc.scalar.activation(hout[:, d, :], psums[d], AF.Relu,
                                         bias=bi_sb[:, d:d + 1],
                                         scale=sc_sb[:, d:d + 1])
                return hout

            h1 = dense_relu_bn(comb, W1, _chunks(D3), sc1_sb, bi1_sb)
            h2 = dense_relu_bn(h1, W2, _chunks(H), sc2_sb, bi2_sb)

            # ---- W3 -> t3 (+b3), two dout groups of 6; fp32r LN stats ----
            t3 = pt([128, _chunks(D3), CS], F32, tag="big12", bufs=2)
            psum_sum = ps.tile([1, CS], F32, tag="mm", name="psum_sum")
            psum_sq = ps.tile([1, CS], F32, tag="mm", name="psum_sq")
            for dg in range(2):
                psums = [mmtile("pw3") for _ in range(6)]
                for k in range(_chunks(H)):
                    ah, al = split_act(h2[:, k, :])
                    wh, wl = load_w_pair(W3, k, 768 * dg, 768)
                    for d in range(6):
                        mm3(psums[d], wh, wl, ah, al, d,
                            start=(k == 0), stop=(k == _chunks(H) - 1))
                for d in range(6):
                    dd = 6 * dg + d
                    nc.scalar.activation(t3[:, dd, :], psums[d], AF.Identity,
                                         bias=b3_sb[:, dd:dd + 1], scale=1.0)
                    t3r = pt([128, CS], F32R, tag="t3r", bufs=2)
                    nc.scalar.copy(t3r, t3[:, dd, :])
                    sqr = pt([128, CS], F32R, tag="sqr", bufs=2)
                    nc.scalar.activation(sqr, t3[:, dd, :], AF.Square)
                    nc.tensor.matmul(psum_sum, ones_r, t3r,
                                     start=(dd == 0), stop=(dd == _chunks(D3) - 1))
                    nc.tensor.matmul(psum_sq, ones_r, sqr,
                                     start=(dd == 0), stop=(dd == _chunks(D3) - 1))

            # ---- LN stats -> r, mean*r; broadcast across partitions ----
            msq = pt([1, CS], F32, tag="st1", bufs=4)
            nc.scalar.activation(msq, psum_sum, AF.Square, scale=1.0 / D3)
            e2p = pt([1, CS], F32, tag="st1", bufs=4)
            nc.scalar.activation(e2p, psum_sq, AF.Copy, bias=EPS, scale=1.0 / D3)
            mean_sb = pt([1, CS], F32, tag="st1", bufs=4)
            nc.scalar.activation(mean_sb, psum_sum, AF.Identity, scale=1.0 / D3)
            veps = pt([1, CS], F32, tag="st1", bufs=4)
            nc.vector.scalar_tensor_tensor(out=veps, in0=msq, scalar=-1.0, in1=e2p,
                                           op0=ALU.mult, op1=ALU.add)
            sdev = pt([1, CS], F32, tag="st1", bufs=4)
            nc.scalar.activation(sdev, veps, AF.Sqrt)
            r_sb = pt([1, CS], F32, tag="st1", bufs=4)
            nc.vector.reciprocal(r_sb, sdev)
            mr_sb = pt([1, CS], F32, tag="st1", bufs=4)
            nc.vector.tensor_mul(mr_sb, mean_sb, r_sb)
            r_r = pt([1, CS], F32R, tag="st1r", bufs=2)
            nc.scalar.copy(r_r, r_sb)
            mr_r = pt([1, CS], F32R, tag="st1r", bufs=2)
            nc.scalar.copy(mr_r, mr_sb)
            psum_rb = mmtile("psum_rb")
            nc.tensor.matmul(psum_rb, onesr_r, r_r, start=True, stop=True)
            rbc = pt([128, CS], F32, tag="bcast", bufs=2)
            nc.scalar.copy(rbc, psum_rb)
            psum_mrb = mmtile("psum_mrb")
            nc.tensor.matmul(psum_mrb, onesr_r, mr_r, start=True, stop=True)
            mrbc = pt([128, CS], F32, tag="bcast", bufs=2)
            nc.scalar.copy(mrbc, psum_mrb)

            # ---- normalize in place: t3 <- LN(t3) =: m ; bf16 copy ----
            m = t3
            for k in range(_chunks(D3)):
                nc.vector.tensor_mul(t3[:, k, :], t3[:, k, :], rbc)
                nc.vector.tensor_sub(t3[:, k, :], t3[:, k, :], mrbc)
                nc.scalar.activation(m[:, k, :], t3[:, k, :], AF.Identity,
                                     bias=lnb_sb[:, k:k + 1], scale=lng_sb[:, k:k + 1])

            # ---- MoE on the three parts + folded output accumulation ----
            outacc = pt([1, CS], F32, tag="outacc", bufs=1)
            for j in range(3):
                o_part = pt([128, 4, CS], BF16, tag="opart", bufs=2)
                mbfp = pt([128, 4, CS], BF16, tag="mbfp", bufs=2)
                for k in range(4):
                    nc.scalar.copy(mbfp[:, k, :], m[:, 4 * j + k, :])
                pend = None
                for c in range(NC_TOK):
                    cs_ = slice(128 * c, 128 * (c + 1))
                    # router logits (fp32, exact routing)
                    psum_log = ps.tile([128, E], F32, tag="mm", name="psum_log")
                    for k in range(4):
                        nc.tensor.matmul(psum_log, m[:, 4 * j + k, cs_],
                                         Wg_sb[:, k, :], start=(k == 0), stop=(k == 3))
                    logits = pt([128, E], F32, tag="logits", bufs=2)
                    nc.vector.tensor_add(logits, psum_log, bg_bc)
                    # top-2 + softmax weights per token
                    max1 = pt([128, 1], F32, tag="max1", bufs=2)
                    nc.vector.reduce_max(max1, logits, axis=AX.X)
                    is1 = pt([128, E], F32, tag="is1", bufs=2)
                    nc.vector.tensor_scalar(out=is1, in0=logits, scalar1=max1,
                                            scalar2=None, op0=ALU.is_equal)
                    l2 = pt([128, E], F32, tag="l2", bufs=2)
                    nc.vector.scalar_tensor_tensor(out=l2, in0=is1, scalar=NEG_BIG,
                                                   in1=logits, op0=ALU.mult,
                                                   op1=ALU.add)
                    max2 = pt([128, 1], F32, tag="max2", bufs=2)
                    nc.vector.reduce_max(max2, l2, axis=AX.X)
                    dlt = pt([128, 1], F32, tag="dlt", bufs=2)
                    nc.vector.tensor_sub(dlt, max1, max2)
                    s1 = pt([128, 1], F32, tag="s1", bufs=2)
                    nc.scalar.activation(s1, dlt, AF.Sigmoid)
                    s2 = pt([128, 1], F32, tag="s2", bufs=2)
                    nc.scalar.activation(s2, dlt, AF.Sigmoid, scale=-1.0)
                    is2 = pt([128, E], F32, tag="is2", bufs=2)
                    nc.vector.tensor_scalar(out=is2, in0=l2, scalar1=max2,
                                            scalar2=None, op0=ALU.is_equal)
                    w_sb = pt([128, E], F32, tag="w_sb", bufs=2)
                    nc.vector.tensor_scalar(out=w_sb, in0=is1, scalar1=s1,
                                            scalar2=None, op0=ALU.mult)
                    nc.vector.scalar_tensor_tensor(out=w_sb, in0=is2, scalar=s2,
                                                   in1=w_sb, op0=ALU.mult, op1=ALU.add)
                    # dense experts in four groups of 2, combined token-major;
                    # o_sb starts from the expert-bias term (w @ bexp)
                    o_sb = pt([128, CS], F32, tag="o_sb", bufs=2)
                    for g in range(4):
                        eps_ = [mmtile("pe") for _ in range(2)]
                        for k in range(4):
                            lhsT = mbfp[:, k, cs_]
                            for ei in range(2):
                                e = 2 * g + ei
                                nc.tensor.matmul(eps_[ei], lhsT, We_sb[:, e, k, :],
                                                 start=(k == 0), stop=(k == 3))
                        if g == 0:
                            # wT + bias matmul while DVE finishes w
                            psum_wt = ps.tile([E, 128], F32, tag="mm", name="psum_wt")
                            nc.tensor.transpose(psum_wt, w_sb, ident_sb)
                            wT_bf = pt([E, 128], BF16, tag="wT_bf", bufs=2)
                            nc.scalar.copy(wT_bf, psum_wt)
                            psum_b = mmtile("psum_b")
                            nc.tensor.matmul(psum_b, wT_bf, bexp_sb,
                                             start=True, stop=True)
                            nc.scalar.copy(o_sb, psum_b)
                        for ei in range(2):
                            e = 2 * g + ei
                            nc.vector.scalar_tensor_tensor(
                                out=o_sb, in0=eps_[ei], scalar=w_sb[:, e:e + 1],
                                in1=o_sb, op0=ALU.mult, op1=ALU.add)
                        if g == 1 and pend is not None:
                            # pipelined: transpose the PREVIOUS chunk's output
                            po, pc = pend
                            psum_ot = mmtile("psum_ot")
                            for d in range(4):
                                nc.tensor.transpose(
                                    psum_ot[:, 128 * d:128 * (d + 1)],
                                    po[:, 128 * d:128 * (d + 1)], ident_sb)
                            nc.scalar.copy(
                                o_part[:, :, slice(128 * pc, 128 * (pc + 1))],
                                psum_ot.rearrange("p (d c) -> p d c", d=4))
                            pend = None
                    pend = (o_sb, c)
                # drain the last chunk's transpose
                po, pc = pend
                psum_ot = mmtile("psum_ot")
                for d in range(4):
                    nc.tensor.transpose(psum_ot[:, 128 * d:128 * (d + 1)],
                                        po[:, 128 * d:128 * (d + 1)], ident_sb)
                nc.scalar.copy(o_part[:, :, slice(128 * pc, 128 * (pc + 1))],
                               psum_ot.rearrange("p (d c) -> p d c", d=4))
                # folded output: outacc += o_part @ Wfr[part j]
                psum_oj = ps.tile([1, CS], F32, tag="mm", name="psum_oj")
                for k in range(4):
                    nc.tensor.matmul(psum_oj, Wfr_sb[:, 4 * j + k, :],
                                     o_part[:, k, :], start=(k == 0), stop=(k == 3))
                if j == 0:
                    nc.scalar.copy(outacc, psum_oj)
                else:
                    nc.vector.tensor_add(outacc, outacc, psum_oj)
            orow = pt([1, CS], F32, tag="orow", bufs=1)
            nc.scalar.activation(orow, outacc, AF.Identity, bias=c0_sb, scale=1.0)
            nc.sync.dma_start(out=out_d[:, ts], in_=orow)

    nc.compile()
    return nc


def _pack_vec(v, nch):
    return np.ascontiguousarray(v.reshape(nch, 128).T.astype(np.float32))


def _tf32_split(w):
    """Split fp32 matrix into tf32-representable hi + lo (RNE at 11
    mantissa bits, matching the PE's fp32r rounding)."""
    w = np.ascontiguousarray(w, np.float32)

    def rnd(x):
        u = x.view(np.uint32)
        keep = ((u + 0x800 + ((u >> 12) & 1)) & 0xFFFFF000).astype(np.uint32)
        return keep.view(np.float32)

    hi = rnd(w)
    lo = rnd((w.astype(np.float64) - hi.astype(np.float64)).astype(np.float32))
    return hi, lo


def prepare_maps(inputs):
    """Host-side sharding + weight prep. Returns per-core input maps."""
    f32 = np.float32
    k64 = 1.0 / np.sqrt(np.float64(1.0) + np.float64(EPS))
    k = f32(k64)
    g1 = inputs["g1"].astype(f32)
    g2 = inputs["g2"].astype(f32)
    # folded output vector: out = concat(o) @ (Wf @ (scf*Wr)) + c0
    scf64 = inputs["bng"].astype(np.float64) * k64
    wfr64 = inputs["Wf"].astype(np.float64) @ (scf64 * inputs["Wr"][:, 0].astype(np.float64))
    c064 = (float(np.dot(inputs["bf"].astype(np.float64) * scf64
                         + inputs["bnb"].astype(np.float64),
                         inputs["Wr"][:, 0].astype(np.float64)))
            + float(inputs["br"][0]))
    consts = {
        "sc1": _pack_vec(g1 * k, _chunks(H)),
        "bi1": _pack_vec(inputs["b1"] * g1 * k + inputs["be1"], _chunks(H)),
        "sc2": _pack_vec(g2 * k, _chunks(H)),
        "bi2": _pack_vec(inputs["b2"] * g2 * k + inputs["be2"], _chunks(H)),
        "b3v": _pack_vec(inputs["b3"], _chunks(D3)),
        "lngv": _pack_vec(inputs["lng"], _chunks(D3)),
        "lnbv": _pack_vec(inputs["lnb"], _chunks(D3)),
        "Wg_r": np.ascontiguousarray(
            inputs["Wg"].reshape(_chunks(D), 128, E).transpose(1, 0, 2), f32),
        "Wfr_r": np.ascontiguousarray(
            wfr64.astype(f32).reshape(_chunks(D3), 128, 1).transpose(1, 0, 2)
        ).astype(ml_dtypes.bfloat16),
        "We_r": np.ascontiguousarray(
            inputs["We"].reshape(E, _chunks(D), 128, D).transpose(2, 0, 1, 3)
        ).astype(ml_dtypes.bfloat16),
        "bexp_bf": np.ascontiguousarray(inputs["bexp"]).astype(ml_dtypes.bfloat16),
        "bg_v": np.ascontiguousarray(inputs["bg"], f32).reshape(1, E),
        "c0_v": np.full((1, 1), c064, f32),
        "ones_col": np.ones((128, 1), f32),
        "ones_row": np.ones((1, 128), f32),
        "ident": np.eye(128, dtype=f32),
    }
    for nm in ["W1", "W2", "W3"]:
        consts[nm] = np.ascontiguousarray(inputs[nm], f32)
    for i in range(3):
        consts[f"Wp{i+1}"] = np.ascontiguousarray(inputs[f"Wp{i+1}"], f32)
        consts[f"bp{i+1}"] = _pack_vec(inputs[f"bp{i+1}"], _chunks(D))
    xts = [np.ascontiguousarray(inputs[f"x{i+1}"].astype(f32).T) for i in range(3)]
    in_maps = []
    for c in range(N_CORES):
        m = dict(consts)
        sl = slice(c * TOK_CORE, (c + 1) * TOK_CORE)
        for i in range(3):
            m[f"x{i+1}t"] = np.ascontiguousarray(xts[i][:, sl])
        in_maps.append(m)
    return in_maps


def run(inputs, trace=False, n_tok=TOK_CORE):
    key = n_tok
    if key not in _PROGRAM_CACHE:
        _PROGRAM_CACHE[key] = build_program(n_tok=n_tok)
    nc = _PROGRAM_CACHE[key]
    in_maps = prepare_maps(inputs)
    res = run_bass_kernel_spmd(nc, in_maps, list(range(N_CORES)), trace=trace)
    rows = [res.results[c]["out"][0] for c in range(N_CORES)]
    out = np.concatenate(rows).reshape(B, 1).astype(np.float32)
    return out, res


def kernel(**inputs):
    out, _ = run(inputs, trace=False)
    return out



# revision 5
# speedup vs baseline: 1.5826x; 1.5826x over previous
"""Trainium2 Bass kernel for nn_CombinedMLPMoEModel (moe_routing).

Strategy (8 NeuronCores, pure data parallel on the batch):
 - Host: shard batch 16384 -> 8 x 2048 tokens, pre-transpose x1/x2/x3 to
   feature-major [Din, tok]; replicate weights.
 - The final output is a scalar per token: concat(o1,o2,o3) @ Wf -> bn
   -> @ Wr.  That tail is linear, so each MoE expert's contribution
   collapses to a per-token SCALAR:
       o_j . wfr_j = sum_k g_k (m_j . (W_e @ wfr_j) + b_e . wfr_j)
   with wfr = Wf @ (bn_scale * Wr).  The dense [512x512] expert matmuls
   disappear; per part we need one [512 -> 8] matmul (like the router),
   computed in exact fp32.
 - LayerNorm is linear per token, so it folds into those matmuls:
       logits = r*(t3 @ (lng.Wg)) - (mu*r)*c_g + bias_g     (same for S)
   i.e. no normalized tensor is ever materialized; the per-token affine
   (r, mu*r) is applied on [128,48] tiles.
 - The proj layer folds into W1 on the host (WF_i = Wp_i @ W1_i; same
   FLOPs, one less pipeline stage).
 - MLP chain precision: fp32r (tf32-like) 3-term split
       W @ x ~= Whi@xhi + Whi@xlo + Wlo@xhi
   with Whi/Wlo PRE-SPLIT ON THE HOST (doubles weight DMA, removes the
   on-chip weight-split engine work).  Routing (top-2 of 8) then matches
   the fp32 reference exactly (0 flips on the reference inputs; total
   rel err ~1e-5).
 - LN stats token-major: sum(t3) rides as a free ones-column (col 48) of
   the z-matmul; sum(t3^2) via a squares tensor against a ones vector.
"""

import numpy as np
from contextlib import ExitStack

import concourse.bass as bass
from concourse import bacc
import concourse.mybir as mybir
import concourse.tile as tile
from concourse.bass_utils import run_bass_kernel_spmd

F32 = mybir.dt.float32
F32R = mybir.dt.float32r
AF = mybir.ActivationFunctionType
ALU = mybir.AluOpType
AX = mybir.AxisListType

N_CORES = 8
B = 16384
DIN = 1024
D = 512
H = 1024
D3 = 3 * D          # 1536
E = 8
TOK_CORE = B // N_CORES   # 2048
MT = 512                  # megatile tokens
EPS = 1e-5
NEG_BIG = -1.0e30

_PROGRAM_CACHE = {}


def _chunks(n):
    return n // 128


def build_program(n_tok=TOK_CORE, mt=MT):
    """Build the per-core Bass program (SPMD across the 8 cores)."""
    nc = bacc.Bacc(None, target_bir_lowering=False)
    T = n_tok // mt
    CS = mt
    NCH = _chunks(mt)          # 128-token chunks per megatile
    NCOL = T * NCH             # output columns per core

    # ---------------- DRAM I/O ----------------
    xs = [nc.dram_tensor(f"x{i+1}t", [DIN, n_tok], F32, kind="ExternalInput")
          for i in range(3)]
    WFhi = nc.dram_tensor("WFhi", [3 * DIN, H], F32R, kind="ExternalInput")
    WFlo = nc.dram_tensor("WFlo", [3 * DIN, H], F32R, kind="ExternalInput")
    W2hi = nc.dram_tensor("W2hi", [H, H], F32R, kind="ExternalInput")
    W2lo = nc.dram_tensor("W2lo", [H, H], F32R, kind="ExternalInput")
    W3hi = nc.dram_tensor("W3hi", [H, D3], F32R, kind="ExternalInput")
    W3lo = nc.dram_tensor("W3lo", [H, D3], F32R, kind="ExternalInput")
    s1d = nc.dram_tensor("s1v", [128, _chunks(H)], F32, kind="ExternalInput")
    b1d = nc.dram_tensor("b1v", [128, _chunks(H)], F32, kind="ExternalInput")
    s2d = nc.dram_tensor("s2v", [128, _chunks(H)], F32, kind="ExternalInput")
    b2d = nc.dram_tensor("b2v", [128, _chunks(H)], F32, kind="ExternalInput")
    b3d = nc.dram_tensor("b3v", [128, _chunks(D3)], F32, kind="ExternalInput")
    wzd = nc.dram_tensor("wz", [128, _chunks(D3), 49], F32, kind="ExternalInput")
    cnegd = nc.dram_tensor("cneg", [1, 48], F32, kind="ExternalInput")
    bzd = nc.dram_tensor("bz", [1, 48], F32, kind="ExternalInput")
    onesd = nc.dram_tensor("ones_col", [128, 1], F32, kind="ExternalInput")
    out_d = nc.dram_tensor("out", [128, NCOL], F32, kind="ExternalOutput")

    with tile.TileContext(nc) as tc, ExitStack() as ctx:
        cp = ctx.enter_context(tc.tile_pool(name="consts", bufs=1))
        sp = ctx.enter_context(tc.tile_pool(name="work", bufs=1))
        ps = ctx.enter_context(tc.tile_pool(name="psum", bufs=8, space="PSUM"))

        def pt(shape, dtype, tag, bufs=None):
            return sp.tile(shape, dtype, tag=tag, bufs=bufs, name=tag)

        # ---------------- resident constants ----------------
        wz_sb = cp.tile([128, _chunks(D3), 49], F32, name="wz_sb")
        nc.sync.dma_start(out=wz_sb, in_=wzd[:, :, :])
        ones_sb = cp.tile([128, 1], F32, name="ones_sb")
        nc.gpsimd.dma_start(out=ones_sb, in_=onesd[:, :])
        cneg_bc = cp.tile([128, 48], F32, name="cneg_bc")
        nc.gpsimd.dma_start(
            out=cneg_bc,
            in_=bass.AP(tensor=cnegd[:, :].tensor, offset=0, ap=[[0, 128], [1, 48]]),
        )
        bz_bc = cp.tile([128, 48], F32, name="bz_bc")
        nc.gpsimd.dma_start(
            out=bz_bc,
            in_=bass.AP(tensor=bzd[:, :].tensor, offset=0, ap=[[0, 128], [1, 48]]),
        )

        def ldvec(dram, nch, name):
            t = cp.tile([128, nch], F32, name=name)
            nc.sync.dma_start(out=t, in_=dram[:, :])
            return t

        s1_sb = ldvec(s1d, _chunks(H), "s1_sb")
        b1_sb = ldvec(b1d, _chunks(H), "b1_sb")
        s2_sb = ldvec(s2d, _chunks(H), "s2_sb")
        b2_sb = ldvec(b2d, _chunks(H), "b2_sb")
        b3_sb = ldvec(b3d, _chunks(D3), "b3_sb")

        out128 = cp.tile([128, NCOL], F32, name="out128")

        def split_act(src_ap):
            """tf32 hi/lo split of one [128, CS] fp32 activation chunk."""
            hi = pt([128, CS], F32R, tag="aph", bufs=2)
            nc.scalar.copy(hi, src_ap)
            lo = pt([128, CS], F32R, tag="apl", bufs=2)
            nc.vector.scalar_tensor_tensor(out=lo, in0=src_ap, scalar=-1.0,
                                           in1=hi.bitcast(F32), op0=ALU.bypass,
                                           op1=ALU.subtract)
            return hi, lo

        def load_w(hi_dram, lo_dram, r0, dgs, dgw):
            """DMA pre-split tf32 hi/lo weight chunk (rows [r0, r0+128),
            dout cols [dgs, dgs+dgw))."""
            wh = pt([128, 1024], F32R, tag="wkh", bufs=3)[:, :dgw]
            nc.sync.dma_start(out=wh, in_=hi_dram[r0:r0 + 128, dgs:dgs + dgw])
            wl = pt([128, 1024], F32R, tag="wkl", bufs=3)[:, :dgw]
            nc.sync.dma_start(out=wl, in_=lo_dram[r0:r0 + 128, dgs:dgs + dgw])
            return wh, wl

        def mm3(psum, wh, wl, xh, xl, d, start, stop):
            sl = slice(128 * d, 128 * (d + 1))
            nc.tensor.matmul(psum, wh[:, sl], xh, start=start, stop=False)
            nc.tensor.matmul(psum, wh[:, sl], xl, start=False, stop=False)
            nc.tensor.matmul(psum, wl[:, sl], xh, start=False, stop=stop)

        # ---------------- megatile loop ----------------
        for t in range(T):
            ts = slice(t * CS, (t + 1) * CS)

            # ---- W1F: h1 = relu(bn(sum_i x_i @ WF_i + b1')) ----
            h1 = pt([128, _chunks(H), CS], F32, tag="h1", bufs=1)
            psums = [ps.tile([128, 512], F32, tag="mm", name="p1") for _ in range(8)]
            for i in range(3):
                for k in range(_chunks(DIN)):
                    kc = _chunks(DIN) * i + k
                    xk = pt([128, CS], F32, tag="xk", bufs=3)
                    nc.sync.dma_start(out=xk, in_=xs[i][128 * k:128 * (k + 1), ts])
                    xh, xl = split_act(xk)
                    wh, wl = load_w(WFhi, WFlo, 128 * kc, 0, 1024)
                    for d in range(8):
                        mm3(psums[d], wh, wl, xh, xl, d,
                            start=(kc == 0), stop=(kc == 3 * _chunks(DIN) - 1))
            for d in range(8):
                nc.scalar.activation(h1[:, d, :], psums[d], AF.Relu,
                                     bias=b1_sb[:, d:d + 1], scale=s1_sb[:, d:d + 1])

            # ---- W2 -> h2 ----
            h2 = pt([128, _chunks(H), CS], F32, tag="h2", bufs=1)
            psums = [ps.tile([128, 512], F32, tag="mm", name="p2") for _ in range(8)]
            for k in range(_chunks(H)):
                ah, al = split_act(h1[:, k, :])
                wh, wl = load_w(W2hi, W2lo, 128 * k, 0, 1024)
                for d in range(8):
                    mm3(psums[d], wh, wl, ah, al, d,
                        start=(k == 0), stop=(k == _chunks(H) - 1))
            for d in range(8):
                nc.scalar.activation(h2[:, d, :], psums[d], AF.Relu,
                                     bias=b2_sb[:, d:d + 1], scale=s2_sb[:, d:d + 1])

            # ---- W3 -> t3 (+b3) and squares, two dout groups of 6 ----
            t3 = pt([128, _chunks(D3), CS], F32, tag="t3", bufs=2)
            sq = pt([128, _chunks(D3), CS], F32, tag="sq", bufs=1)
            for dg in range(2):
                psums = [ps.tile([128, 512], F32, tag="mm", name="p3")
                         for _ in range(6)]
                for k in range(_chunks(H)):
                    ah, al = split_act(h2[:, k, :])
                    wh, wl = load_w(W3hi, W3lo, 128 * k, 768 * dg, 768)
                    for d in range(6):
                        mm3(psums[d], wh, wl, ah, al, d,
                            start=(k == 0), stop=(k == _chunks(H) - 1))
                for d in range(6):
                    dd = 6 * dg + d
                    nc.scalar.activation(t3[:, dd, :], psums[d], AF.Identity,
                                         bias=b3_sb[:, dd:dd + 1], scale=1.0)
                    nc.scalar.activation(sq[:, dd, :], psums[d], AF.Square,
                                         bias=b3_sb[:, dd:dd + 1], scale=1.0)

            # ---- tail: router+expert scalars + LN affine, per 128-chunk ----
            for c in range(NCH):
                cs_ = slice(128 * c, 128 * (c + 1))
                pz = ps.tile([128, 49], F32, tag="mm", name="pz")
                pq = ps.tile([128, 1], F32, tag="mm", name="pq")
                for kc in range(_chunks(D3)):
                    nc.tensor.matmul(pz, t3[:, kc, cs_], wz_sb[:, kc, :],
                                     start=(kc == 0), stop=(kc == _chunks(D3) - 1))
                for kc in range(_chunks(D3)):
                    nc.tensor.matmul(pq, sq[:, kc, cs_], ones_sb,
                                     start=(kc == 0), stop=(kc == _chunks(D3) - 1))

                # per-token LN stats (token-major [128,1])
                mu = pt([128, 1], F32, tag="mu", bufs=2)
                nc.vector.tensor_scalar(out=mu, in0=pz[:, 48:49],
                                        scalar1=1.0 / D3, scalar2=None, op0=ALU.mult)
                et2 = pt([128, 1], F32, tag="et2", bufs=2)
                nc.vector.tensor_scalar(out=et2, in0=pq, scalar1=1.0 / D3,
                                        scalar2=EPS, op0=ALU.mult, op1=ALU.add)
                msq = pt([128, 1], F32, tag="msq", bufs=2)
                nc.scalar.activation(msq, mu, AF.Square)
                veps = pt([128, 1], F32, tag="veps", bufs=2)
                nc.vector.scalar_tensor_tensor(out=veps, in0=msq, scalar=-1.0,
                                               in1=et2, op0=ALU.mult, op1=ALU.add)
                sdev = pt([128, 1], F32, tag="sdev", bufs=2)
                nc.scalar.activation(sdev, veps, AF.Sqrt)
                r_t = pt([128, 1], F32, tag="r_t", bufs=2)
                nc.vector.reciprocal(r_t, sdev)
                mr_t = pt([128, 1], F32, tag="mr_t", bufs=2)
                nc.vector.tensor_mul(mr_t, mu, r_t)

                # z = r*Z - (mu*r)*c + bias   on [128,48]
                z = pt([128, 48], F32, tag="z", bufs=2)
                nc.vector.tensor_scalar(out=z, in0=pz[:, 0:48], scalar1=r_t,
                                        scalar2=None, op0=ALU.mult)
                nc.vector.scalar_tensor_tensor(out=z, in0=cneg_bc, scalar=mr_t,
                                               in1=z, op0=ALU.mult, op1=ALU.add)
                nc.vector.tensor_add(z, z, bz_bc)

                # per part: top-2 softmax gates, then sum_e w_e * S_e
                ctbs = []
                for j in range(3):
                    lg = z[:, 16 * j:16 * j + 8]
                    Sv = z[:, 16 * j + 8:16 * j + 16]
                    max1 = pt([128, 1], F32, tag="max1", bufs=2)
                    nc.vector.reduce_max(max1, lg, axis=AX.X)
                    is1 = pt([128, 8], F32, tag="is1", bufs=2)
                    nc.vector.tensor_scalar(out=is1, in0=lg, scalar1=max1,
                                            scalar2=None, op0=ALU.is_equal)
                    l2 = pt([128, 8], F32, tag="l2", bufs=2)
                    nc.vector.scalar_tensor_tensor(out=l2, in0=is1, scalar=NEG_BIG,
                                                   in1=lg, op0=ALU.mult, op1=ALU.add)
                    max2 = pt([128, 1], F32, tag="max2", bufs=2)
                    nc.vector.reduce_max(max2, l2, axis=AX.X)
                    dlt = pt([128, 1], F32, tag="dlt", bufs=2)
                    nc.vector.tensor_sub(dlt, max1, max2)
                    s1 = pt([128, 1], F32, tag="s1", bufs=2)
                    nc.scalar.activation(s1, dlt, AF.Sigmoid)
                    s2 = pt([128, 1], F32, tag="s2", bufs=2)
                    nc.scalar.activation(s2, dlt, AF.Sigmoid, scale=-1.0)
                    is2 = pt([128, 8], F32, tag="is2", bufs=2)
                    nc.vector.tensor_scalar(out=is2, in0=l2, scalar1=max2,
                                            scalar2=None, op0=ALU.is_equal)
                    w_sb = pt([128, 8], F32, tag="w_sb", bufs=2)
                    nc.vector.tensor_scalar(out=w_sb, in0=is1, scalar1=s1,
                                            scalar2=None, op0=ALU.mult)
                    nc.vector.scalar_tensor_tensor(out=w_sb, in0=is2, scalar=s2,
                                                   in1=w_sb, op0=ALU.mult,
                                                   op1=ALU.add)
                    wS = pt([128, 8], F32, tag="wS", bufs=2)
                    ctb = pt([128, 1], F32, tag="ctb", bufs=3)
                    nc.vector.scalar_tensor_tensor(out=wS, in0=Sv, scalar=1.0,
                                                   in1=w_sb, op0=ALU.bypass,
                                                   op1=ALU.mult, accum_out=ctb)
                    ctbs.append(ctb)

                col = NCH * t + c
                c01 = pt([128, 1], F32, tag="c01", bufs=2)
                nc.vector.tensor_add(c01, ctbs[0], ctbs[1])
                nc.vector.tensor_add(out128[:, col:col + 1], c01, ctbs[2])

        nc.sync.dma_start(out=out_d[:, :], in_=out128)

    nc.compile()
    return nc


def _pack_vec(v, nch):
    return np.ascontiguousarray(np.asarray(v, np.float32).reshape(nch, 128).T)


def _tf32_split(w):
    """Split fp32 matrix into tf32-representable hi + lo (RNE at 11
    mantissa bits, matching the PE's fp32r rounding)."""
    w = np.ascontiguousarray(w, np.float32)

    def rnd(x):
        u = x.view(np.uint32)
        keep = ((u + 0x800 + ((u >> 12) & 1)) & 0xFFFFF000).astype(np.uint32)
        return keep.view(np.float32)

    hi = rnd(w)
    lo = rnd((w.astype(np.float64) - hi.astype(np.float64)).astype(np.float32))
    return hi, lo


def prepare_maps(inputs):
    """Host-side sharding + weight folding. Returns per-core input maps
    plus the global output constant c0."""
    f32, f64 = np.float32, np.float64
    k64 = 1.0 / np.sqrt(f64(1.0) + f64(EPS))
    g1 = np.asarray(inputs["g1"], f64)
    g2 = np.asarray(inputs["g2"], f64)

    # ---- fold proj into W1: WF_i = Wp_i @ W1_i ; b1' = sum_i bp_i@W1_i + b1
    W1 = np.asarray(inputs["W1"], f64)
    WF = np.concatenate(
        [np.asarray(inputs[f"Wp{i+1}"], f64) @ W1[D * i:D * (i + 1), :]
         for i in range(3)], axis=0)                        # [3*DIN, H]
    b1p = (np.concatenate([np.asarray(inputs[f"bp{i+1}"], f64)
                           for i in range(3)]) @ W1
           + np.asarray(inputs["b1"], f64))

    # ---- output-tail fold: out = concat(o) @ wfr + c0
    scf = np.asarray(inputs["bng"], f64) * k64
    wfr = np.asarray(inputs["Wf"], f64) @ (scf * np.asarray(inputs["Wr"], f64)[:, 0])
    c0 = ((np.asarray(inputs["bf"], f64) * scf + np.asarray(inputs["bnb"], f64))
          @ np.asarray(inputs["Wr"], f64)[:, 0] + f64(inputs["br"][0]))

    # ---- LN fold into router / expert-scalar weights
    lng = np.asarray(inputs["lng"], f64)
    lnb = np.asarray(inputs["lnb"], f64)
    Wg = np.asarray(inputs["Wg"], f64)
    bg = np.asarray(inputs["bg"], f64)
    We = np.asarray(inputs["We"], f64)
    bexp = np.asarray(inputs["bexp"], f64)
    wzfull = np.zeros((D3, 49), f64)
    cneg = np.zeros(48, f64)
    bz = np.zeros(48, f64)
    for j in range(3):
        sl = slice(D * j, D * (j + 1))
        lngj, lnbj, wfrj = lng[sl], lnb[sl], wfr[sl]
        Vj = (We @ wfrj).T                                  # [D, E]
        wzfull[sl, 16 * j:16 * j + 8] = lngj[:, None] * Wg
        wzfull[sl, 16 * j + 8:16 * j + 16] = lngj[:, None] * Vj
        cneg[16 * j:16 * j + 8] = -(lngj @ Wg)
        cneg[16 * j + 8:16 * j + 16] = -(lngj @ Vj)
        bz[16 * j:16 * j + 8] = bg + lnbj @ Wg
        bz[16 * j + 8:16 * j + 16] = bexp @ wfrj + lnbj @ Vj
    wzfull[:, 48] = 1.0

    WFhi, WFlo = _tf32_split(WF.astype(f32))
    W2hi, W2lo = _tf32_split(inputs["W2"])
    W3hi, W3lo = _tf32_split(inputs["W3"])
    consts = {
        "WFhi": WFhi, "WFlo": WFlo,
        "W2hi": W2hi, "W2lo": W2lo,
        "W3hi": W3hi, "W3lo": W3lo,
        "s1v": _pack_vec((g1 * k64).astype(f32), _chunks(H)),
        "b1v": _pack_vec((b1p * g1 * k64
                          + np.asarray(inputs["be1"], f64)).astype(f32), _chunks(H)),
        "s2v": _pack_vec((g2 * k64).astype(f32), _chunks(H)),
        "b2v": _pack_vec((np.asarray(inputs["b2"], f64) * g2 * k64
                          + np.asarray(inputs["be2"], f64)).astype(f32), _chunks(H)),
        "b3v": _pack_vec(inputs["b3"], _chunks(D3)),
        "wz": np.ascontiguousarray(
            wzfull.astype(f32).reshape(_chunks(D3), 128, 49).transpose(1, 0, 2)),
        "cneg": cneg.astype(f32).reshape(1, 48),
        "bz": bz.astype(f32).reshape(1, 48),
        "ones_col": np.ones((128, 1), f32),
    }
    xts = [np.ascontiguousarray(np.asarray(inputs[f"x{i+1}"], f32).T)
           for i in range(3)]
    in_maps = []
    for c in range(N_CORES):
        m = dict(consts)
        sl = slice(c * TOK_CORE, (c + 1) * TOK_CORE)
        for i in range(3):
            m[f"x{i+1}t"] = np.ascontiguousarray(xts[i][:, sl])
        in_maps.append(m)
    return in_maps, c0


def run(inputs, trace=False, n_tok=TOK_CORE):
    key = n_tok
    if key not in _PROGRAM_CACHE:
        _PROGRAM_CACHE[key] = build_program(n_tok=n_tok)
    nc = _PROGRAM_CACHE[key]
    in_maps, c0 = prepare_maps(inputs)
    res = run_bass_kernel_spmd(nc, in_maps, list(range(N_CORES)), trace=trace)
    rows = []
    for c in range(N_CORES):
        arr = res.results[c]["out"]            # [128, NCOL]; token = col*128 + row
        rows.append(np.ascontiguousarray(arr.T).reshape(-1))
    out = (np.concatenate(rows).astype(np.float64) + c0).astype(np.float32)
    return out.reshape(B, 1), res


def kernel(**inputs):
    out, _ = run(inputs, trace=False)
    return out


# revision 13
# speedup vs baseline: 1.7648x; 1.1151x over previous
"""Trainium2 Bass kernel for nn_CombinedMLPMoEModel (moe_routing).

Strategy (8 NeuronCores, pure data parallel on the batch):
 - Host: shard batch 16384 -> 8 x 2048 tokens, pre-transpose x1/x2/x3 to
   feature-major [Din, tok]; replicate weights.
 - The final output is a scalar per token: concat(o1,o2,o3) @ Wf -> bn
   -> @ Wr.  That tail is linear, so each MoE expert's contribution
   collapses to a per-token SCALAR:
       o_j . wfr_j = sum_k g_k (m_j . (W_e @ wfr_j) + b_e . wfr_j)
   with wfr = Wf @ (bn_scale * Wr).  The dense [512x512] expert matmuls
   disappear; per part we need one [512 -> 8] matmul (like the router),
   computed in exact fp32.
 - LayerNorm is linear per token, so it folds into those matmuls:
       logits = r*(t3 @ (lng.Wg)) - (mu*r)*c_g + bias_g     (same for S)
   i.e. no normalized tensor is ever materialized; the per-token affine
   (r, mu*r) is applied on [128,48] tiles.
 - The proj layer folds into W1 on the host (WF_i = Wp_i @ W1_i; same
   FLOPs, one less pipeline stage).
 - MLP chain precision: fp32r (tf32-like) 3-term split
       W @ x ~= Whi@xhi + Whi@xlo + Wlo@xhi
   with Whi/Wlo PRE-SPLIT ON THE HOST (doubles weight DMA, removes the
   on-chip weight-split engine work).  Routing (top-2 of 8) then matches
   the fp32 reference exactly (0 flips on the reference inputs; total
   rel err ~1e-5).
 - LN stats token-major: sum(t3) rides as a free ones-column (col 48) of
   the z-matmul; sum(t3^2) via a squares tensor against a ones vector.
"""

import numpy as np
from contextlib import ExitStack

import concourse.bass as bass
from concourse import bacc
import concourse.mybir as mybir
import concourse.tile as tile
from concourse.bass_utils import run_bass_kernel_spmd

F32 = mybir.dt.float32
F32R = mybir.dt.float32r
AF = mybir.ActivationFunctionType
ALU = mybir.AluOpType
AX = mybir.AxisListType

N_CORES = 8
B = 16384
DIN = 1024
D = 512
H = 1024
D3 = 3 * D          # 1536
E = 8
TOK_CORE = B // N_CORES   # 2048
MT = 512                  # megatile tokens
EPS = 1e-5
NEG_BIG = -1.0e30

_PROGRAM_CACHE = {}


def _chunks(n):
    return n // 128


def build_program(n_tok=TOK_CORE, mt=MT):
    """Build the per-core Bass program (SPMD across the 8 cores)."""
    nc = bacc.Bacc(None, target_bir_lowering=False)
    T = n_tok // mt
    CS = mt
    NCH = _chunks(mt)          # 128-token chunks per megatile
    NCOL = T * NCH             # output columns per core

    # ---------------- DRAM I/O ----------------
    xs = [nc.dram_tensor(f"x{i+1}t", [DIN, n_tok], F32, kind="ExternalInput")
          for i in range(3)]
    WFhi = nc.dram_tensor("WFhi", [3 * DIN, H], F32R, kind="ExternalInput")
    WFlo = nc.dram_tensor("WFlo", [3 * DIN, H], F32R, kind="ExternalInput")
    W2hi = nc.dram_tensor("W2hi", [H, H], F32R, kind="ExternalInput")
    W3hi = nc.dram_tensor("W3hi", [H, D3], F32R, kind="ExternalInput")
    s1d = nc.dram_tensor("s1v", [128, _chunks(H)], F32, kind="ExternalInput")
    b1d = nc.dram_tensor("b1v", [128, _chunks(H)], F32, kind="ExternalInput")
    s2d = nc.dram_tensor("s2v", [128, _chunks(H)], F32, kind="ExternalInput")
    b2d = nc.dram_tensor("b2v", [128, _chunks(H)], F32, kind="ExternalInput")
    b3d = nc.dram_tensor("b3v", [128, _chunks(D3)], F32, kind="ExternalInput")
    wzd = nc.dram_tensor("wz", [128, _chunks(D3), 49], F32, kind="ExternalInput")
    wzfd = nc.dram_tensor("wzfix", [128, _chunks(H), 49], F32, kind="ExternalInput")
    cnegd = nc.dram_tensor("cneg", [1, 48], F32, kind="ExternalInput")
    bzd = nc.dram_tensor("bz", [1, 48], F32, kind="ExternalInput")
    onesd = nc.dram_tensor("ones_col", [128, 1], F32, kind="ExternalInput")
    out_d = nc.dram_tensor("out", [128, NCOL], F32, kind="ExternalOutput")

    with tile.TileContext(nc) as tc, ExitStack() as ctx:
        cp = ctx.enter_context(tc.tile_pool(name="consts", bufs=1))
        sp = ctx.enter_context(tc.tile_pool(name="work", bufs=1))
        ps = ctx.enter_context(tc.tile_pool(name="psum", bufs=8, space="PSUM"))

        def pt(shape, dtype, tag, bufs=None):
            return sp.tile(shape, dtype, tag=tag, bufs=bufs, name=tag)

        # ---------------- resident constants ----------------
        wz_sb = cp.tile([128, _chunks(D3), 49], F32, name="wz_sb")
        nc.sync.dma_start(out=wz_sb, in_=wzd[:, :, :])
        wzf_sb = cp.tile([128, _chunks(H), 49], F32, name="wzf_sb")
        nc.sync.dma_start(out=wzf_sb, in_=wzfd[:, :, :])
        ones_sb = cp.tile([128, 1], F32, name="ones_sb")
        nc.gpsimd.dma_start(out=ones_sb, in_=onesd[:, :])
        cneg_bc = cp.tile([128, 48], F32, name="cneg_bc")
        nc.gpsimd.dma_start(
            out=cneg_bc,
            in_=bass.AP(tensor=cnegd[:, :].tensor, offset=0, ap=[[0, 128], [1, 48]]),
        )
        bz_bc = cp.tile([128, 48], F32, name="bz_bc")
        nc.gpsimd.dma_start(
            out=bz_bc,
            in_=bass.AP(tensor=bzd[:, :].tensor, offset=0, ap=[[0, 128], [1, 48]]),
        )

        def ldvec(dram, nch, name):
            t = cp.tile([128, nch], F32, name=name)
            nc.sync.dma_start(out=t, in_=dram[:, :])
            return t

        s1_sb = ldvec(s1d, _chunks(H), "s1_sb")
        b1_sb = ldvec(b1d, _chunks(H), "b1_sb")
        s2_sb = ldvec(s2d, _chunks(H), "s2_sb")
        b2_sb = ldvec(b2d, _chunks(H), "b2_sb")
        b3_sb = ldvec(b3d, _chunks(D3), "b3_sb")

        out128 = cp.tile([128, NCOL], F32, name="out128")

        def split_act(src_ap):
            """tf32 hi/lo split of one [128, CS] fp32 activation chunk."""
            hi = pt([128, CS], F32R, tag="aph", bufs=2)
            nc.scalar.copy(hi, src_ap)
            lo = pt([128, CS], F32R, tag="apl", bufs=2)
            nc.vector.scalar_tensor_tensor(out=lo, in0=src_ap, scalar=-1.0,
                                           in1=hi.bitcast(F32), op0=ALU.bypass,
                                           op1=ALU.subtract)
            return hi, lo

        def load_wh(hi_dram, r0, dgs, dgw):
            """DMA pre-split tf32 hi weight chunk (rows [r0, r0+128),
            dout cols [dgs, dgs+dgw))."""
            wh = pt([128, 1024], F32R, tag="wkh", bufs=3)[:, :dgw]
            nc.sync.dma_start(out=wh, in_=hi_dram[r0:r0 + 128, dgs:dgs + dgw])
            return wh

        def load_wl(lo_dram, r0, dgs, dgw):
            wl = pt([128, 1024], F32R, tag="wkl", bufs=3)[:, :dgw]
            nc.sync.dma_start(out=wl, in_=lo_dram[r0:r0 + 128, dgs:dgs + dgw])
            return wl

        def mm3(psum, wh, wl, xh, xl, d, start, stop):
            sl = slice(128 * d, 128 * (d + 1))
            nc.tensor.matmul(psum, wh[:, sl], xh, start=start, stop=False)
            nc.tensor.matmul(psum, wh[:, sl], xl, start=False, stop=False)
            nc.tensor.matmul(psum, wl[:, sl], xh, start=False, stop=stop)

        def mm2(psum, wh, xh, xl, d, start, stop):
            sl = slice(128 * d, 128 * (d + 1))
            nc.tensor.matmul(psum, wh[:, sl], xh, start=start, stop=False)
            nc.tensor.matmul(psum, wh[:, sl], xl, start=False, stop=stop)

        # ---------------- megatile loop ----------------
        for t in range(T):
            ts = slice(t * CS, (t + 1) * CS)

            # ---- W1F: h1 = relu(bn(sum_i x_i @ WF_i + b1')) ----
            h1 = pt([128, _chunks(H), CS], F32, tag="h1", bufs=1)
            psums = [ps.tile([128, 512], F32, tag="mm", name="p1") for _ in range(8)]
            for i in range(3):
                for k in range(_chunks(DIN)):
                    kc = _chunks(DIN) * i + k
                    xk = pt([128, CS], F32, tag="xk", bufs=3)
                    nc.sync.dma_start(out=xk, in_=xs[i][128 * k:128 * (k + 1), ts])
                    xh, xl = split_act(xk)
                    wh = load_wh(WFhi, 128 * kc, 0, 1024)
                    wl = load_wl(WFlo, 128 * kc, 0, 1024)
                    for d in range(8):
                        mm3(psums[d], wh, wl, xh, xl, d,
                            start=(kc == 0), stop=(kc == 3 * _chunks(DIN) - 1))
            for d in range(8):
                nc.scalar.activation(h1[:, d, :], psums[d], AF.Relu,
                                     bias=b1_sb[:, d:d + 1], scale=s1_sb[:, d:d + 1])

            # ---- W2 -> h2 ----
            h2 = pt([128, _chunks(H), CS], F32, tag="h2", bufs=1)
            psums = [ps.tile([128, 512], F32, tag="mm", name="p2") for _ in range(8)]
            for k in range(_chunks(H)):
                ah, al = split_act(h1[:, k, :])
                wh = load_wh(W2hi, 128 * k, 0, 1024)
                for d in range(8):
                    mm2(psums[d], wh, ah, al, d,
                        start=(k == 0), stop=(k == _chunks(H) - 1))
            for d in range(8):
                nc.scalar.activation(h2[:, d, :], psums[d], AF.Relu,
                                     bias=b2_sb[:, d:d + 1], scale=s2_sb[:, d:d + 1])

            # ---- W3 -> t3 (+b3), 2-pass; squares accumulated into sqacc ----
            t3 = pt([128, _chunks(D3), CS], F32, tag="t3", bufs=2)
            sqacc = pt([128, CS], F32, tag="sqa", bufs=2)
            for dg in range(2):
                psums = [ps.tile([128, 512], F32, tag="mm", name="p3")
                         for _ in range(6)]
                for k in range(_chunks(H)):
                    ah, al = split_act(h2[:, k, :])
                    wh = load_wh(W3hi, 128 * k, 768 * dg, 768)
                    for d in range(6):
                        mm2(psums[d], wh, ah, al, d,
                            start=(k == 0), stop=(k == _chunks(H) - 1))
                for d in range(6):
                    dd = 6 * dg + d
                    nc.scalar.activation(t3[:, dd, :], psums[d], AF.Identity,
                                         bias=b3_sb[:, dd:dd + 1], scale=1.0)
                    if dd == 0:
                        nc.scalar.activation(sqacc, psums[d], AF.Square,
                                             bias=b3_sb[:, dd:dd + 1], scale=1.0)
                    else:
                        sqt = pt([128, CS], F32, tag="sqt", bufs=2)
                        nc.scalar.activation(sqt, psums[d], AF.Square,
                                             bias=b3_sb[:, dd:dd + 1], scale=1.0)
                        nc.vector.tensor_add(sqacc, sqacc, sqt)

            # ---- tail: router+expert scalars + LN affine, per 128-chunk ----
            for c in range(NCH):
                cs_ = slice(128 * c, 128 * (c + 1))
                pz = ps.tile([128, 49], F32, tag="mm", name="pz")
                pq = ps.tile([128, 1], F32, tag="mm", name="pq")
                for kc in range(_chunks(D3)):
                    nc.tensor.matmul(pz, t3[:, kc, cs_], wz_sb[:, kc, :],
                                     start=(kc == 0), stop=False)
                # exact correction of W3's dropped lo-term: h2 @ (W3lo @ wz)
                for k in range(_chunks(H)):
                    nc.tensor.matmul(pz, h2[:, k, cs_], wzf_sb[:, k, :],
                                     start=False, stop=(k == _chunks(H) - 1))
                nc.tensor.matmul(pq, sqacc[:, cs_], ones_sb,
                                 start=True, stop=True)

                # per-token LN stats (token-major [128,1])
                mu = pt([128, 1], F32, tag="mu", bufs=2)
                nc.vector.tensor_scalar(out=mu, in0=pz[:, 48:49],
                                        scalar1=1.0 / D3, scalar2=None, op0=ALU.mult)
                et2 = pt([128, 1], F32, tag="et2", bufs=2)
                nc.vector.tensor_scalar(out=et2, in0=pq, scalar1=1.0 / D3,
                                        scalar2=EPS, op0=ALU.mult, op1=ALU.add)
                msq = pt([128, 1], F32, tag="msq", bufs=2)
                nc.scalar.activation(msq, mu, AF.Square)
                veps = pt([128, 1], F32, tag="veps", bufs=2)
                nc.vector.scalar_tensor_tensor(out=veps, in0=msq, scalar=-1.0,
                                               in1=et2, op0=ALU.mult, op1=ALU.add)
                sdev = pt([128, 1], F32, tag="sdev", bufs=2)
                nc.scalar.activation(sdev, veps, AF.Sqrt)
                r_t = pt([128, 1], F32, tag="r_t", bufs=2)
                nc.vector.reciprocal(r_t, sdev)
                mr_t = pt([128, 1], F32, tag="mr_t", bufs=2)
                nc.vector.tensor_mul(mr_t, mu, r_t)

                # z = r*Z - (mu*r)*c + bias   on [128,48]
                z = pt([128, 48], F32, tag="z", bufs=2)
                nc.vector.tensor_scalar(out=z, in0=pz[:, 0:48], scalar1=r_t,
                                        scalar2=None, op0=ALU.mult)
                nc.vector.scalar_tensor_tensor(out=z, in0=cneg_bc, scalar=mr_t,
                                               in1=z, op0=ALU.mult, op1=ALU.add)
                nc.vector.tensor_add(z, z, bz_bc)

                # per part: top-2 softmax gates, then sum_e w_e * S_e
                ctbs = []
                for j in range(3):
                    lg = z[:, 16 * j:16 * j + 8]
                    Sv = z[:, 16 * j + 8:16 * j + 16]
                    max1 = pt([128, 1], F32, tag="max1", bufs=2)
                    nc.vector.reduce_max(max1, lg, axis=AX.X)
                    is1 = pt([128, 8], F32, tag="is1", bufs=2)
                    nc.vector.tensor_scalar(out=is1, in0=lg, scalar1=max1,
                                            scalar2=None, op0=ALU.is_equal)
                    l2 = pt([128, 8], F32, tag="l2", bufs=2)
                    nc.vector.scalar_tensor_tensor(out=l2, in0=is1, scalar=NEG_BIG,
                                                   in1=lg, op0=ALU.mult, op1=ALU.add)
                    max2 = pt([128, 1], F32, tag="max2", bufs=2)
                    nc.vector.reduce_max(max2, l2, axis=AX.X)
                    dlt = pt([128, 1], F32, tag="dlt", bufs=2)
                    nc.vector.tensor_sub(dlt, max1, max2)
                    s1 = pt([128, 1], F32, tag="s1", bufs=2)
                    nc.scalar.activation(s1, dlt, AF.Sigmoid)
                    s2 = pt([128, 1], F32, tag="s2", bufs=2)
                    nc.scalar.activation(s2, dlt, AF.Sigmoid, scale=-1.0)
                    is2 = pt([128, 8], F32, tag="is2", bufs=2)
                    nc.vector.tensor_scalar(out=is2, in0=l2, scalar1=max2,
                                            scalar2=None, op0=ALU.is_equal)
                    w_sb = pt([128, 8], F32, tag="w_sb", bufs=2)
                    nc.vector.tensor_scalar(out=w_sb, in0=is1, scalar1=s1,
                                            scalar2=None, op0=ALU.mult)
                    nc.vector.scalar_tensor_tensor(out=w_sb, in0=is2, scalar=s2,
                                                   in1=w_sb, op0=ALU.mult,
                                                   op1=ALU.add)
                    wS = pt([128, 8], F32, tag="wS", bufs=2)
                    ctb = pt([128, 1], F32, tag="ctb", bufs=3)
                    nc.vector.scalar_tensor_tensor(out=wS, in0=Sv, scalar=1.0,
                                                   in1=w_sb, op0=ALU.bypass,
                                                   op1=ALU.mult, accum_out=ctb)
                    ctbs.append(ctb)

                col = NCH * t + c
                c01 = pt([128, 1], F32, tag="c01", bufs=2)
                nc.vector.tensor_add(c01, ctbs[0], ctbs[1])
                nc.vector.tensor_add(out128[:, col:col + 1], c01, ctbs[2])

        nc.sync.dma_start(out=out_d[:, :], in_=out128)

    nc.compile()
    return nc


def _pack_vec(v, nch):
    return np.ascontiguousarray(np.asarray(v, np.float32).reshape(nch, 128).T)


def _tf32_split(w):
    """Split fp32 matrix into tf32-representable hi + lo (RNE at 11
    mantissa bits, matching the PE's fp32r rounding)."""
    w = np.ascontiguousarray(w, np.float32)

    def rnd(x):
        u = x.view(np.uint32)
        keep = ((u + 0x800 + ((u >> 12) & 1)) & 0xFFFFF000).astype(np.uint32)
        return keep.view(np.float32)

    hi = rnd(w)
    lo = rnd((w.astype(np.float64) - hi.astype(np.float64)).astype(np.float32))
    return hi, lo


def prepare_maps(inputs):
    """Host-side sharding + weight folding. Returns per-core input maps
    plus the global output constant c0."""
    f32, f64 = np.float32, np.float64
    k64 = 1.0 / np.sqrt(f64(1.0) + f64(EPS))
    g1 = np.asarray(inputs["g1"], f64)
    g2 = np.asarray(inputs["g2"], f64)

    # ---- fold proj into W1: WF_i = Wp_i @ W1_i ; b1' = sum_i bp_i@W1_i + b1
    W1 = np.asarray(inputs["W1"], f64)
    WF = np.concatenate(
        [np.asarray(inputs[f"Wp{i+1}"], f64) @ W1[D * i:D * (i + 1), :]
         for i in range(3)], axis=0)                        # [3*DIN, H]
    b1p = (np.concatenate([np.asarray(inputs[f"bp{i+1}"], f64)
                           for i in range(3)]) @ W1
           + np.asarray(inputs["b1"], f64))

    # ---- output-tail fold: out = concat(o) @ wfr + c0
    scf = np.asarray(inputs["bng"], f64) * k64
    wfr = np.asarray(inputs["Wf"], f64) @ (scf * np.asarray(inputs["Wr"], f64)[:, 0])
    c0 = ((np.asarray(inputs["bf"], f64) * scf + np.asarray(inputs["bnb"], f64))
          @ np.asarray(inputs["Wr"], f64)[:, 0] + f64(inputs["br"][0]))

    # ---- LN fold into router / expert-scalar weights
    lng = np.asarray(inputs["lng"], f64)
    lnb = np.asarray(inputs["lnb"], f64)
    Wg = np.asarray(inputs["Wg"], f64)
    bg = np.asarray(inputs["bg"], f64)
    We = np.asarray(inputs["We"], f64)
    bexp = np.asarray(inputs["bexp"], f64)
    wzfull = np.zeros((D3, 49), f64)
    cneg = np.zeros(48, f64)
    bz = np.zeros(48, f64)
    for j in range(3):
        sl = slice(D * j, D * (j + 1))
        lngj, lnbj, wfrj = lng[sl], lnb[sl], wfr[sl]
        Vj = (We @ wfrj).T                                  # [D, E]
        wzfull[sl, 16 * j:16 * j + 8] = lngj[:, None] * Wg
        wzfull[sl, 16 * j + 8:16 * j + 16] = lngj[:, None] * Vj
        cneg[16 * j:16 * j + 8] = -(lngj @ Wg)
        cneg[16 * j + 8:16 * j + 16] = -(lngj @ Vj)
        bz[16 * j:16 * j + 8] = bg + lnbj @ Wg
        bz[16 * j + 8:16 * j + 16] = bexp @ wfrj + lnbj @ Vj
    wzfull[:, 48] = 1.0

    WFhi, WFlo = _tf32_split(WF.astype(f32))
    W2hi, _ = _tf32_split(inputs["W2"])
    W3hi, _ = _tf32_split(inputs["W3"])
    # exact correction of W3's 2-pass dropped term, folded into z:
    #   zcorr = h2 @ ((W3 - W3hi) @ wz)
    W3lo64 = np.asarray(inputs["W3"], f64) - W3hi.astype(f64)
    wzfix = np.zeros((H, 49), f64)
    wzfix[:, 0:48] = W3lo64 @ wzfull[:, 0:48]
    consts = {
        "WFhi": WFhi, "WFlo": WFlo,
        "W2hi": W2hi,
        "W3hi": W3hi,
        "wzfix": np.ascontiguousarray(
            wzfix.astype(f32).reshape(_chunks(H), 128, 49).transpose(1, 0, 2)),
        "s1v": _pack_vec((g1 * k64).astype(f32), _chunks(H)),
        "b1v": _pack_vec((b1p * g1 * k64
                          + np.asarray(inputs["be1"], f64)).astype(f32), _chunks(H)),
        "s2v": _pack_vec((g2 * k64).astype(f32), _chunks(H)),
        "b2v": _pack_vec((np.asarray(inputs["b2"], f64) * g2 * k64
                          + np.asarray(inputs["be2"], f64)).astype(f32), _chunks(H)),
        "b3v": _pack_vec(inputs["b3"], _chunks(D3)),
        "wz": np.ascontiguousarray(
            wzfull.astype(f32).reshape(_chunks(D3), 128, 49).transpose(1, 0, 2)),
        "cneg": cneg.astype(f32).reshape(1, 48),
        "bz": bz.astype(f32).reshape(1, 48),
        "ones_col": np.ones((128, 1), f32),
    }
    xts = [np.ascontiguousarray(np.asarray(inputs[f"x{i+1}"], f32).T)
           for i in range(3)]
    in_maps = []
    for c in range(N_CORES):
        m = dict(consts)
        sl = slice(c * TOK_CORE, (c + 1) * TOK_CORE)
        for i in range(3):
            m[f"x{i+1}t"] = np.ascontiguousarray(xts[i][:, sl])
        in_maps.append(m)
    return in_maps, c0


def run(inputs, trace=False, n_tok=TOK_CORE):
    key = n_tok
    if key not in _PROGRAM_CACHE:
        _PROGRAM_CACHE[key] = build_program(n_tok=n_tok)
    nc = _PROGRAM_CACHE[key]
    in_maps, c0 = prepare_maps(inputs)
    res = run_bass_kernel_spmd(nc, in_maps, list(range(N_CORES)), trace=trace)
    rows = []
    for c in range(N_CORES):
        arr = res.results[c]["out"]            # [128, NCOL]; token = col*128 + row
        rows.append(np.ascontiguousarray(arr.T).reshape(-1))
    out = (np.concatenate(rows).astype(np.float64) + c0).astype(np.float32)
    return out.reshape(B, 1), res


def kernel(**inputs):
    out, _ = run(inputs, trace=False)
    return out


# revision 18
# speedup vs baseline: 2.0219x; 1.1457x over previous
"""Trainium2 Bass kernel for nn_CombinedMLPMoEModel (moe_routing).

Strategy (8 NeuronCores, pure data parallel on the batch):
 - Host: shard batch 16384 -> 8 x 2048 tokens, pre-transpose x1/x2/x3 to
   feature-major [Din, tok]; replicate weights.
 - The final output is a scalar per token: concat(o1,o2,o3) @ Wf -> bn
   -> @ Wr.  That tail is linear, so each MoE expert's contribution
   collapses to a per-token SCALAR:
       o_j . wfr_j = sum_k g_k (m_j . (W_e @ wfr_j) + b_e . wfr_j)
   with wfr = Wf @ (bn_scale * Wr).  The dense [512x512] expert matmuls
   disappear; per part we need one [512 -> 8] matmul (like the router),
   computed in exact fp32.
 - LayerNorm is linear per token, so it folds into those matmuls:
       logits = r*(t3 @ (lng.Wg)) - (mu*r)*c_g + bias_g     (same for S)
   i.e. no normalized tensor is ever materialized; the per-token affine
   (r, mu*r) is applied on [128,48] tiles.
 - The proj layer folds into W1 on the host (WF_i = Wp_i @ W1_i; same
   FLOPs, one less pipeline stage).
 - MLP chain precision: fp32r (tf32-like) 3-term split
       W @ x ~= Whi@xhi + Whi@xlo + Wlo@xhi
   with Whi/Wlo PRE-SPLIT ON THE HOST (doubles weight DMA, removes the
   on-chip weight-split engine work).  Routing (top-2 of 8) then matches
   the fp32 reference exactly (0 flips on the reference inputs; total
   rel err ~1e-5).
 - LN stats token-major: sum(t3) rides as a free ones-column (col 48) of
   the z-matmul; sum(t3^2) via a squares tensor against a ones vector.
"""

import numpy as np
from contextlib import ExitStack

import concourse.bass as bass
from concourse import bacc
import concourse.mybir as mybir
import concourse.tile as tile
from concourse.bass_utils import run_bass_kernel_spmd

F32 = mybir.dt.float32
F32R = mybir.dt.float32r
AF = mybir.ActivationFunctionType
ALU = mybir.AluOpType
AX = mybir.AxisListType

N_CORES = 8
B = 16384
DIN = 1024
D = 512
H = 1024
D3 = 3 * D          # 1536
E = 8
TOK_CORE = B // N_CORES   # 2048
MT = 512                  # megatile tokens
EPS = 1e-5
NEG_BIG = -1.0e30

_PROGRAM_CACHE = {}


def _chunks(n):
    return n // 128


def build_program(n_tok=TOK_CORE, mt=MT):
    """Build the per-core Bass program (SPMD across the 8 cores)."""
    nc = bacc.Bacc(None, target_bir_lowering=False)
    T = n_tok // mt
    CS = mt
    NCH = _chunks(mt)          # 128-token chunks per megatile
    NCOL = T * NCH             # output columns per core

    # ---------------- DRAM I/O ----------------
    xs = [nc.dram_tensor(f"x{i+1}t", [DIN, n_tok], F32, kind="ExternalInput")
          for i in range(3)]
    WFhi = nc.dram_tensor("WFhi", [3 * DIN, H], F32R, kind="ExternalInput")
    WFlo = nc.dram_tensor("WFlo", [3 * DIN, H], F32R, kind="ExternalInput")
    W2hi = nc.dram_tensor("W2hi", [H, H], F32R, kind="ExternalInput")
    W3hi = nc.dram_tensor("W3hi", [H, D3], F32R, kind="ExternalInput")
    s1d = nc.dram_tensor("s1v", [128, _chunks(H)], F32, kind="ExternalInput")
    b1d = nc.dram_tensor("b1v", [128, _chunks(H)], F32, kind="ExternalInput")
    s2d = nc.dram_tensor("s2v", [128, _chunks(H)], F32, kind="ExternalInput")
    b2d = nc.dram_tensor("b2v", [128, _chunks(H)], F32, kind="ExternalInput")
    b3d = nc.dram_tensor("b3v", [128, _chunks(D3)], F32, kind="ExternalInput")
    wzhd = nc.dram_tensor("wzh", [128, _chunks(H), 49], F32, kind="ExternalInput")
    zbd = nc.dram_tensor("zb", [1, 49], F32, kind="ExternalInput")
    onesrd = nc.dram_tensor("ones_row", [1, 128], F32, kind="ExternalInput")
    cnegd = nc.dram_tensor("cneg", [1, 48], F32, kind="ExternalInput")
    bzd = nc.dram_tensor("bz", [1, 48], F32, kind="ExternalInput")
    onesd = nc.dram_tensor("ones_col", [128, 1], F32, kind="ExternalInput")
    out_d = nc.dram_tensor("out", [128, NCOL], F32, kind="ExternalOutput")

    with tile.TileContext(nc) as tc, ExitStack() as ctx:
        cp = ctx.enter_context(tc.tile_pool(name="consts", bufs=1))
        sp = ctx.enter_context(tc.tile_pool(name="work", bufs=1))
        ps = ctx.enter_context(tc.tile_pool(name="psum", bufs=8, space="PSUM"))

        def pt(shape, dtype, tag, bufs=None):
            return sp.tile(shape, dtype, tag=tag, bufs=bufs, name=tag)

        # ---------------- resident constants ----------------
        wzh_sb = cp.tile([128, _chunks(H), 49], F32, name="wzh_sb")
        nc.sync.dma_start(out=wzh_sb, in_=wzhd[:, :, :])
        zb_sb = cp.tile([1, 49], F32, name="zb_sb")
        nc.sync.dma_start(out=zb_sb, in_=zbd[:, :])
        onesr_sb = cp.tile([1, 128], F32, name="onesr_sb")
        nc.sync.dma_start(out=onesr_sb, in_=onesrd[:, :])
        ones_sb = cp.tile([128, 1], F32, name="ones_sb")
        nc.gpsimd.dma_start(out=ones_sb, in_=onesd[:, :])
        cneg_bc = cp.tile([128, 48], F32, name="cneg_bc")
        nc.gpsimd.dma_start(
            out=cneg_bc,
            in_=bass.AP(tensor=cnegd[:, :].tensor, offset=0, ap=[[0, 128], [1, 48]]),
        )
        bz_bc = cp.tile([128, 48], F32, name="bz_bc")
        nc.gpsimd.dma_start(
            out=bz_bc,
            in_=bass.AP(tensor=bzd[:, :].tensor, offset=0, ap=[[0, 128], [1, 48]]),
        )

        def ldvec(dram, nch, name):
            t = cp.tile([128, nch], F32, name=name)
            nc.sync.dma_start(out=t, in_=dram[:, :])
            return t

        s1_sb = ldvec(s1d, _chunks(H), "s1_sb")
        b1_sb = ldvec(b1d, _chunks(H), "b1_sb")
        s2_sb = ldvec(s2d, _chunks(H), "s2_sb")
        b2_sb = ldvec(b2d, _chunks(H), "b2_sb")
        b3_sb = ldvec(b3d, _chunks(D3), "b3_sb")

        out128 = cp.tile([128, NCOL], F32, name="out128")

        def split_act(src_ap):
            """tf32 hi/lo split of one [128, CS] fp32 activation chunk."""
            hi = pt([128, CS], F32R, tag="aph", bufs=2)
            nc.scalar.copy(hi, src_ap)
            lo = pt([128, CS], F32R, tag="apl", bufs=2)
            nc.vector.scalar_tensor_tensor(out=lo, in0=src_ap, scalar=-1.0,
                                           in1=hi.bitcast(F32), op0=ALU.bypass,
                                           op1=ALU.subtract)
            return hi, lo

        def load_wh(hi_dram, r0, dgs, dgw):
            """DMA pre-split tf32 hi weight chunk (rows [r0, r0+128),
            dout cols [dgs, dgs+dgw))."""
            wh = pt([128, 1024], F32R, tag="wkh", bufs=3)[:, :dgw]
            nc.sync.dma_start(out=wh, in_=hi_dram[r0:r0 + 128, dgs:dgs + dgw])
            return wh

        def load_wl(lo_dram, r0, dgs, dgw):
            wl = pt([128, 1024], F32R, tag="wkl", bufs=3)[:, :dgw]
            nc.sync.dma_start(out=wl, in_=lo_dram[r0:r0 + 128, dgs:dgs + dgw])
            return wl

        def mm3(psum, wh, wl, xh, xl, d, start, stop):
            sl = slice(128 * d, 128 * (d + 1))
            nc.tensor.matmul(psum, wh[:, sl], xh, start=start, stop=False)
            nc.tensor.matmul(psum, wh[:, sl], xl, start=False, stop=False)
            nc.tensor.matmul(psum, wl[:, sl], xh, start=False, stop=stop)

        def mm2(psum, wh, xh, xl, d, start, stop):
            sl = slice(128 * d, 128 * (d + 1))
            nc.tensor.matmul(psum, wh[:, sl], xh, start=start, stop=False)
            nc.tensor.matmul(psum, wh[:, sl], xl, start=False, stop=stop)

        # ---------------- megatile loop ----------------
        for t in range(T):
            ts = slice(t * CS, (t + 1) * CS)

            # ---- W1F: h1 = relu(bn(sum_i x_i @ WF_i + b1')) ----
            h1 = pt([128, _chunks(H), CS], F32, tag="h1", bufs=1)
            psums = [ps.tile([128, 512], F32, tag="mm", name="p1") for _ in range(8)]
            for i in range(3):
                for k in range(_chunks(DIN)):
                    kc = _chunks(DIN) * i + k
                    xk = pt([128, CS], F32, tag="xk", bufs=3)
                    nc.sync.dma_start(out=xk, in_=xs[i][128 * k:128 * (k + 1), ts])
                    xh, xl = split_act(xk)
                    wh = load_wh(WFhi, 128 * kc, 0, 1024)
                    wl = load_wl(WFlo, 128 * kc, 0, 1024)
                    for d in range(8):
                        mm3(psums[d], wh, wl, xh, xl, d,
                            start=(kc == 0), stop=(kc == 3 * _chunks(DIN) - 1))
            for d in range(8):
                nc.scalar.activation(h1[:, d, :], psums[d], AF.Relu,
                                     bias=b1_sb[:, d:d + 1], scale=s1_sb[:, d:d + 1])

            # ---- W2 -> h2 ----
            h2 = pt([128, _chunks(H), CS], F32, tag="h2", bufs=1)
            psums = [ps.tile([128, 512], F32, tag="mm", name="p2") for _ in range(8)]
            for k in range(_chunks(H)):
                ah, al = split_act(h1[:, k, :])
                wh = load_wh(W2hi, 128 * k, 0, 1024)
                for d in range(8):
                    mm2(psums[d], wh, ah, al, d,
                        start=(k == 0), stop=(k == _chunks(H) - 1))
            for d in range(8):
                nc.scalar.activation(h2[:, d, :], psums[d], AF.Relu,
                                     bias=b2_sb[:, d:d + 1], scale=s2_sb[:, d:d + 1])

            # ---- W3, single pass (feeds LN variance only); squares -> sqacc ----
            ahh = pt([128, _chunks(H), CS], F32R, tag="ahh", bufs=1)
            for k in range(_chunks(H)):
                nc.scalar.copy(ahh[:, k, :], h2[:, k, :])
            sqacc = pt([128, CS], F32, tag="sqa", bufs=2)
            for dg in range(2):
                psums = [ps.tile([128, 512], F32, tag="mm", name="p3")
                         for _ in range(6)]
                for k in range(_chunks(H)):
                    wh = load_wh(W3hi, 128 * k, 768 * dg, 768)
                    for d in range(6):
                        nc.tensor.matmul(psums[d], wh[:, 128 * d:128 * (d + 1)],
                                         ahh[:, k, :],
                                         start=(k == 0), stop=(k == _chunks(H) - 1))
                for d in range(6):
                    dd = 6 * dg + d
                    if dd == 0:
                        nc.scalar.activation(sqacc, psums[d], AF.Square,
                                             bias=b3_sb[:, dd:dd + 1], scale=1.0)
                    else:
                        sqt = pt([128, CS], F32, tag="sqt", bufs=2)
                        nc.scalar.activation(sqt, psums[d], AF.Square,
                                             bias=b3_sb[:, dd:dd + 1], scale=1.0)
                        nc.vector.tensor_add(sqacc, sqacc, sqt)

            # ---- tail: z entirely from h2 (exact W3 fold) per 128-chunk ----
            for c in range(NCH):
                cs_ = slice(128 * c, 128 * (c + 1))
                pz = ps.tile([128, 49], F32, tag="mm", name="pz")
                pq = ps.tile([128, 1], F32, tag="mm", name="pq")
                for k in range(_chunks(H)):
                    nc.tensor.matmul(pz, h2[:, k, cs_], wzh_sb[:, k, :],
                                     start=(k == 0), stop=False)
                # + [b3 @ wz | sum(b3)] broadcast row
                nc.tensor.matmul(pz, onesr_sb, zb_sb, start=False, stop=True)
                nc.tensor.matmul(pq, sqacc[:, cs_], ones_sb,
                                 start=True, stop=True)

                # per-token LN stats (token-major [128,1])
                mu = pt([128, 1], F32, tag="mu", bufs=2)
                nc.vector.tensor_scalar(out=mu, in0=pz[:, 48:49],
                                        scalar1=1.0 / D3, scalar2=None, op0=ALU.mult)
                et2 = pt([128, 1], F32, tag="et2", bufs=2)
                nc.vector.tensor_scalar(out=et2, in0=pq, scalar1=1.0 / D3,
                                        scalar2=EPS, op0=ALU.mult, op1=ALU.add)
                msq = pt([128, 1], F32, tag="msq", bufs=2)
                nc.scalar.activation(msq, mu, AF.Square)
                veps = pt([128, 1], F32, tag="veps", bufs=2)
                nc.vector.scalar_tensor_tensor(out=veps, in0=msq, scalar=-1.0,
                                               in1=et2, op0=ALU.mult, op1=ALU.add)
                sdev = pt([128, 1], F32, tag="sdev", bufs=2)
                nc.scalar.activation(sdev, veps, AF.Sqrt)
                r_t = pt([128, 1], F32, tag="r_t", bufs=2)
                nc.vector.reciprocal(r_t, sdev)
                mr_t = pt([128, 1], F32, tag="mr_t", bufs=2)
                nc.vector.tensor_mul(mr_t, mu, r_t)

                # z = r*Z - (mu*r)*c + bias   on [128,48]
                z = pt([128, 48], F32, tag="z", bufs=2)
                nc.vector.tensor_scalar(out=z, in0=pz[:, 0:48], scalar1=r_t,
                                        scalar2=None, op0=ALU.mult)
                nc.vector.scalar_tensor_tensor(out=z, in0=cneg_bc, scalar=mr_t,
                                               in1=z, op0=ALU.mult, op1=ALU.add)
                nc.vector.tensor_add(z, z, bz_bc)

                # per part: top-2 softmax gates, then sum_e w_e * S_e
                ctbs = []
                for j in range(3):
                    lg = z[:, 16 * j:16 * j + 8]
                    Sv = z[:, 16 * j + 8:16 * j + 16]
                    max1 = pt([128, 1], F32, tag="max1", bufs=2)
                    nc.vector.reduce_max(max1, lg, axis=AX.X)
                    is1 = pt([128, 8], F32, tag="is1", bufs=2)
                    nc.vector.tensor_scalar(out=is1, in0=lg, scalar1=max1,
                                            scalar2=None, op0=ALU.is_equal)
                    l2 = pt([128, 8], F32, tag="l2", bufs=2)
                    nc.vector.scalar_tensor_tensor(out=l2, in0=is1, scalar=NEG_BIG,
                                                   in1=lg, op0=ALU.mult, op1=ALU.add)
                    max2 = pt([128, 1], F32, tag="max2", bufs=2)
                    nc.vector.reduce_max(max2, l2, axis=AX.X)
                    dlt = pt([128, 1], F32, tag="dlt", bufs=2)
                    nc.vector.tensor_sub(dlt, max1, max2)
                    s1 = pt([128, 1], F32, tag="s1", bufs=2)
                    nc.scalar.activation(s1, dlt, AF.Sigmoid)
                    s2 = pt([128, 1], F32, tag="s2", bufs=2)
                    nc.scalar.activation(s2, dlt, AF.Sigmoid, scale=-1.0)
                    is2 = pt([128, 8], F32, tag="is2", bufs=2)
                    nc.vector.tensor_scalar(out=is2, in0=l2, scalar1=max2,
                                            scalar2=None, op0=ALU.is_equal)
                    w_sb = pt([128, 8], F32, tag="w_sb", bufs=2)
                    nc.vector.tensor_scalar(out=w_sb, in0=is1, scalar1=s1,
                                            scalar2=None, op0=ALU.mult)
                    nc.vector.scalar_tensor_tensor(out=w_sb, in0=is2, scalar=s2,
                                                   in1=w_sb, op0=ALU.mult,
                                                   op1=ALU.add)
                    wS = pt([128, 8], F32, tag="wS", bufs=2)
                    ctb = pt([128, 1], F32, tag="ctb", bufs=3)
                    nc.vector.scalar_tensor_tensor(out=wS, in0=Sv, scalar=1.0,
                                                   in1=w_sb, op0=ALU.bypass,
                                                   op1=ALU.mult, accum_out=ctb)
                    ctbs.append(ctb)

                col = NCH * t + c
                c01 = pt([128, 1], F32, tag="c01", bufs=2)
                nc.vector.tensor_add(c01, ctbs[0], ctbs[1])
                nc.vector.tensor_add(out128[:, col:col + 1], c01, ctbs[2])

        nc.sync.dma_start(out=out_d[:, :], in_=out128)

    nc.compile()
    return nc


def _pack_vec(v, nch):
    return np.ascontiguousarray(np.asarray(v, np.float32).reshape(nch, 128).T)


def _tf32_split(w):
    """Split fp32 matrix into tf32-representable hi + lo (RNE at 11
    mantissa bits, matching the PE's fp32r rounding)."""
    w = np.ascontiguousarray(w, np.float32)

    def rnd(x):
        u = x.view(np.uint32)
        keep = ((u + 0x800 + ((u >> 12) & 1)) & 0xFFFFF000).astype(np.uint32)
        return keep.view(np.float32)

    hi = rnd(w)
    lo = rnd((w.astype(np.float64) - hi.astype(np.float64)).astype(np.float32))
    return hi, lo


def prepare_maps(inputs):
    """Host-side sharding + weight folding. Returns per-core input maps
    plus the global output constant c0."""
    f32, f64 = np.float32, np.float64
    k64 = 1.0 / np.sqrt(f64(1.0) + f64(EPS))
    g1 = np.asarray(inputs["g1"], f64)
    g2 = np.asarray(inputs["g2"], f64)

    # ---- fold proj into W1: WF_i = Wp_i @ W1_i ; b1' = sum_i bp_i@W1_i + b1
    W1 = np.asarray(inputs["W1"], f64)
    WF = np.concatenate(
        [np.asarray(inputs[f"Wp{i+1}"], f64) @ W1[D * i:D * (i + 1), :]
         for i in range(3)], axis=0)                        # [3*DIN, H]
    b1p = (np.concatenate([np.asarray(inputs[f"bp{i+1}"], f64)
                           for i in range(3)]) @ W1
           + np.asarray(inputs["b1"], f64))

    # ---- output-tail fold: out = concat(o) @ wfr + c0
    scf = np.asarray(inputs["bng"], f64) * k64
    wfr = np.asarray(inputs["Wf"], f64) @ (scf * np.asarray(inputs["Wr"], f64)[:, 0])
    c0 = ((np.asarray(inputs["bf"], f64) * scf + np.asarray(inputs["bnb"], f64))
          @ np.asarray(inputs["Wr"], f64)[:, 0] + f64(inputs["br"][0]))

    # ---- LN fold into router / expert-scalar weights
    lng = np.asarray(inputs["lng"], f64)
    lnb = np.asarray(inputs["lnb"], f64)
    Wg = np.asarray(inputs["Wg"], f64)
    bg = np.asarray(inputs["bg"], f64)
    We = np.asarray(inputs["We"], f64)
    bexp = np.asarray(inputs["bexp"], f64)
    wzfull = np.zeros((D3, 49), f64)
    cneg = np.zeros(48, f64)
    bz = np.zeros(48, f64)
    for j in range(3):
        sl = slice(D * j, D * (j + 1))
        lngj, lnbj, wfrj = lng[sl], lnb[sl], wfr[sl]
        Vj = (We @ wfrj).T                                  # [D, E]
        wzfull[sl, 16 * j:16 * j + 8] = lngj[:, None] * Wg
        wzfull[sl, 16 * j + 8:16 * j + 16] = lngj[:, None] * Vj
        cneg[16 * j:16 * j + 8] = -(lngj @ Wg)
        cneg[16 * j + 8:16 * j + 16] = -(lngj @ Vj)
        bz[16 * j:16 * j + 8] = bg + lnbj @ Wg
        bz[16 * j + 8:16 * j + 16] = bexp @ wfrj + lnbj @ Vj
    wzfull[:, 48] = 1.0

    WFhi, WFlo = _tf32_split(WF.astype(f32))
    W2hi, _ = _tf32_split(inputs["W2"])
    W3hi, _ = _tf32_split(inputs["W3"])
    # z comes entirely from h2:  Z = h2 @ (W3 @ wz) + b3 @ wz  (exact W3),
    # col 48 is the feature-sum for mu:  sum(t3) = h2 @ W3.sum(1) + sum(b3)
    W3f = np.asarray(inputs["W3"], f64)
    b3f = np.asarray(inputs["b3"], f64)
    wzh = np.zeros((H, 49), f64)
    wzh[:, 0:48] = W3f @ wzfull[:, 0:48]
    wzh[:, 48] = W3f.sum(1)
    zb = np.zeros(49, f64)
    zb[0:48] = b3f @ wzfull[:, 0:48]
    zb[48] = b3f.sum()
    consts = {
        "WFhi": WFhi, "WFlo": WFlo,
        "W2hi": W2hi,
        "W3hi": W3hi,
        "wzh": np.ascontiguousarray(
            wzh.astype(f32).reshape(_chunks(H), 128, 49).transpose(1, 0, 2)),
        "zb": zb.astype(f32).reshape(1, 49),
        "ones_row": np.ones((1, 128), f32),
        "s1v": _pack_vec((g1 * k64).astype(f32), _chunks(H)),
        "b1v": _pack_vec((b1p * g1 * k64
                          + np.asarray(inputs["be1"], f64)).astype(f32), _chunks(H)),
        "s2v": _pack_vec((g2 * k64).astype(f32), _chunks(H)),
        "b2v": _pack_vec((np.asarray(inputs["b2"], f64) * g2 * k64
                          + np.asarray(inputs["be2"], f64)).astype(f32), _chunks(H)),
        "b3v": _pack_vec(inputs["b3"], _chunks(D3)),
        "cneg": cneg.astype(f32).reshape(1, 48),
        "bz": bz.astype(f32).reshape(1, 48),
        "ones_col": np.ones((128, 1), f32),
    }
    xts = [np.ascontiguousarray(np.asarray(inputs[f"x{i+1}"], f32).T)
           for i in range(3)]
    in_maps = []
    for c in range(N_CORES):
        m = dict(consts)
        sl = slice(c * TOK_CORE, (c + 1) * TOK_CORE)
        for i in range(3):
            m[f"x{i+1}t"] = np.ascontiguousarray(xts[i][:, sl])
        in_maps.append(m)
    return in_maps, c0


def run(inputs, trace=False, n_tok=TOK_CORE):
    key = n_tok
    if key not in _PROGRAM_CACHE:
        _PROGRAM_CACHE[key] = build_program(n_tok=n_tok)
    nc = _PROGRAM_CACHE[key]
    in_maps, c0 = prepare_maps(inputs)
    res = run_bass_kernel_spmd(nc, in_maps, list(range(N_CORES)), trace=trace)
    rows = []
    for c in range(N_CORES):
        arr = res.results[c]["out"]            # [128, NCOL]; token = col*128 + row
        rows.append(np.ascontiguousarray(arr.T).reshape(-1))
    out = (np.concatenate(rows).astype(np.float64) + c0).astype(np.float32)
    return out.reshape(B, 1), res


def kernel(**inputs):
    out, _ = run(inputs, trace=False)
    return out


# revision 26
# speedup vs baseline: 2.4520x; 1.2127x over previous
"""Trainium2 Bass kernel for nn_CombinedMLPMoEModel (moe_routing).

Strategy (8 NeuronCores, pure data parallel on the batch):
 - Host: shard batch 16384 -> 8 x 2048 tokens, pre-transpose x1/x2/x3 to
   feature-major [Din, tok]; replicate weights.
 - The final output is a scalar per token: concat(o1,o2,o3) @ Wf -> bn
   -> @ Wr.  That tail is linear, so each MoE expert's contribution
   collapses to a per-token SCALAR:
       o_j . wfr_j = sum_k g_k (m_j . (W_e @ wfr_j) + b_e . wfr_j)
   with wfr = Wf @ (bn_scale * Wr).  The dense [512x512] expert matmuls
   disappear; per part we need one [512 -> 8] matmul (like the router),
   computed in exact fp32.
 - LayerNorm is linear per token, so it folds into those matmuls:
       logits = r*(t3 @ (lng.Wg)) - (mu*r)*c_g + bias_g     (same for S)
   i.e. no normalized tensor is ever materialized; the per-token affine
   (r, mu*r) is applied on [128,48] tiles.
 - The proj layer folds into W1 on the host (WF_i = Wp_i @ W1_i; same
   FLOPs, one less pipeline stage).
 - MLP chain precision: fp32r (tf32-like) 3-term split
       W @ x ~= Whi@xhi + Whi@xlo + Wlo@xhi
   with Whi/Wlo PRE-SPLIT ON THE HOST (doubles weight DMA, removes the
   on-chip weight-split engine work).  Routing (top-2 of 8) then matches
   the fp32 reference exactly (0 flips on the reference inputs; total
   rel err ~1e-5).
 - LN stats token-major: sum(t3) rides as a free ones-column (col 48) of
   the z-matmul; sum(t3^2) via a squares tensor against a ones vector.
"""

import numpy as np
import ml_dtypes
from contextlib import ExitStack

import concourse.bass as bass
from concourse import bacc
import concourse.mybir as mybir
import concourse.tile as tile
from concourse.bass_utils import run_bass_kernel_spmd

F32 = mybir.dt.float32
F32R = mybir.dt.float32r
F8 = mybir.dt.float8e5
DR = mybir.MatmulPerfMode.DoubleRow
AF = mybir.ActivationFunctionType
ALU = mybir.AluOpType
AX = mybir.AxisListType

N_CORES = 8
B = 16384
DIN = 1024
D = 512
H = 1024
D3 = 3 * D          # 1536
E = 8
TOK_CORE = B // N_CORES   # 2048
MT = 512                  # megatile tokens
EPS = 1e-5
NEG_BIG = -1.0e30

_PROGRAM_CACHE = {}


def _chunks(n):
    return n // 128


def build_program(n_tok=TOK_CORE, mt=MT):
    """Build the per-core Bass program (SPMD across the 8 cores)."""
    nc = bacc.Bacc(None, target_bir_lowering=False)
    T = n_tok // mt
    CS = mt
    NCH = _chunks(mt)          # 128-token chunks per megatile
    NCOL = T * NCH             # output columns per core

    # ---------------- DRAM I/O ----------------
    xs = [nc.dram_tensor(f"x{i+1}t", [DIN, n_tok], F32, kind="ExternalInput")
          for i in range(3)]
    WFhi = nc.dram_tensor("WFhi", [3 * DIN, H], F32R, kind="ExternalInput")
    W2hi = nc.dram_tensor("W2hi", [H, H], F32R, kind="ExternalInput")
    W3hi = nc.dram_tensor("W3hi", [H, D3], F32R, kind="ExternalInput")
    # fp8-e5m2 lo-pass weights, packed as [128, kpair, 2, dout]
    WFh8d = nc.dram_tensor("WFh8", [128, 12, 2, H], F8, kind="ExternalInput")
    WFl8d = nc.dram_tensor("WFl8", [128, 12, 2, H], F8, kind="ExternalInput")
    W2h8d = nc.dram_tensor("W2h8", [128, 4, 2, H], F8, kind="ExternalInput")
    s1d = nc.dram_tensor("s1v", [128, _chunks(H)], F32, kind="ExternalInput")
    b1d = nc.dram_tensor("b1v", [128, _chunks(H)], F32, kind="ExternalInput")
    s2d = nc.dram_tensor("s2v", [128, _chunks(H)], F32, kind="ExternalInput")
    b2d = nc.dram_tensor("b2v", [128, _chunks(H)], F32, kind="ExternalInput")
    b3d = nc.dram_tensor("b3v", [128, _chunks(D3)], F32, kind="ExternalInput")
    wzhd = nc.dram_tensor("wzh", [128, _chunks(H), 49], F32, kind="ExternalInput")
    zbd = nc.dram_tensor("zb", [1, 49], F32, kind="ExternalInput")
    onesrd = nc.dram_tensor("ones_row", [1, 128], F32, kind="ExternalInput")
    cnegd = nc.dram_tensor("cneg", [1, 48], F32, kind="ExternalInput")
    bzd = nc.dram_tensor("bz", [1, 48], F32, kind="ExternalInput")
    onesd = nc.dram_tensor("ones_col", [128, 1], F32, kind="ExternalInput")
    out_d = nc.dram_tensor("out", [128, NCOL], F32, kind="ExternalOutput")

    with tile.TileContext(nc) as tc, ExitStack() as ctx:
        cp = ctx.enter_context(tc.tile_pool(name="consts", bufs=1))
        sp = ctx.enter_context(tc.tile_pool(name="work", bufs=1))
        ps = ctx.enter_context(tc.tile_pool(name="psum", bufs=8, space="PSUM"))

        def pt(shape, dtype, tag, bufs=None):
            return sp.tile(shape, dtype, tag=tag, bufs=bufs, name=tag)

        # ---------------- resident constants ----------------
        wzh_sb = cp.tile([128, _chunks(H), 49], F32, name="wzh_sb")
        nc.sync.dma_start(out=wzh_sb, in_=wzhd[:, :, :])
        zb_sb = cp.tile([1, 49], F32, name="zb_sb")
        nc.sync.dma_start(out=zb_sb, in_=zbd[:, :])
        onesr_sb = cp.tile([1, 128], F32, name="onesr_sb")
        nc.sync.dma_start(out=onesr_sb, in_=onesrd[:, :])
        ones_sb = cp.tile([128, 1], F32, name="ones_sb")
        nc.gpsimd.dma_start(out=ones_sb, in_=onesd[:, :])
        cneg_bc = cp.tile([128, 48], F32, name="cneg_bc")
        nc.gpsimd.dma_start(
            out=cneg_bc,
            in_=bass.AP(tensor=cnegd[:, :].tensor, offset=0, ap=[[0, 128], [1, 48]]),
        )
        bz_bc = cp.tile([128, 48], F32, name="bz_bc")
        nc.gpsimd.dma_start(
            out=bz_bc,
            in_=bass.AP(tensor=bzd[:, :].tensor, offset=0, ap=[[0, 128], [1, 48]]),
        )

        def ldvec(dram, nch, name):
            t = cp.tile([128, nch], F32, name=name)
            nc.sync.dma_start(out=t, in_=dram[:, :])
            return t

        s1_sb = ldvec(s1d, _chunks(H), "s1_sb")
        b1_sb = ldvec(b1d, _chunks(H), "b1_sb")
        s2_sb = ldvec(s2d, _chunks(H), "s2_sb")
        b2_sb = ldvec(b2d, _chunks(H), "b2_sb")
        b3_sb = ldvec(b3d, _chunks(D3), "b3_sb")

        out128 = cp.tile([128, NCOL], F32, name="out128")

        def split_act(src_ap):
            """tf32 hi/lo split of one [128, CS] fp32 activation chunk."""
            hi = pt([128, CS], F32R, tag="aph", bufs=2)
            nc.scalar.copy(hi, src_ap)
            lo = pt([128, CS], F32R, tag="apl", bufs=2)
            nc.vector.scalar_tensor_tensor(out=lo, in0=src_ap, scalar=-1.0,
                                           in1=hi.bitcast(F32), op0=ALU.bypass,
                                           op1=ALU.subtract)
            return hi, lo

        def load_wh(hi_dram, r0, dgs, dgw):
            """DMA pre-split tf32 hi weight chunk (rows [r0, r0+128),
            dout cols [dgs, dgs+dgw))."""
            wh = pt([128, 1024], F32R, tag="wkh", bufs=3)[:, :dgw]
            nc.sync.dma_start(out=wh, in_=hi_dram[r0:r0 + 128, dgs:dgs + dgw])
            return wh

        def load_w8(dram8, kp, tag):
            w8 = pt([128, 2, H], F8, tag=tag, bufs=3)
            nc.sync.dma_start(out=w8, in_=dram8[:, kp, :, :])
            return w8

        # ---------------- megatile loop ----------------
        for t in range(T):
            ts = slice(t * CS, (t + 1) * CS)

            # ---- W1F: h1 = relu(bn(sum_i x_i @ WF_i + b1')) ----
            # pass 1: tf32 Whi@xhi per k-chunk; passes 2+3: fp8-e5m2
            # DoubleRow over k-pairs (Whi@xlo and Wlo@xhi, scale-neutral).
            h1 = pt([128, _chunks(H), CS], F32, tag="h1", bufs=1)
            psums = [ps.tile([128, 512], F32, tag="mm", name="p1") for _ in range(8)]
            for kp in range(12):
                xh8 = pt([128, 2, CS], F8, tag="xh8", bufs=2)
                xl8 = pt([128, 2, CS], F8, tag="xl8", bufs=2)
                for kk in range(2):
                    kc = 2 * kp + kk
                    i, k = kc // _chunks(DIN), kc % _chunks(DIN)
                    xk = pt([128, CS], F32, tag="xk", bufs=3)
                    nc.sync.dma_start(out=xk, in_=xs[i][128 * k:128 * (k + 1), ts])
                    xh, xl = split_act(xk)
                    nc.scalar.activation(xh8[:, kk, :], xh.bitcast(F32), AF.Copy,
                                         scale=1.0 / 1024.0)
                    nc.scalar.activation(xl8[:, kk, :], xl.bitcast(F32), AF.Copy,
                                         scale=16.0)
                    wh = load_wh(WFhi, 128 * kc, 0, 1024)
                    for d in range(8):
                        nc.tensor.matmul(psums[d], wh[:, 128 * d:128 * (d + 1)],
                                         xh, start=(kc == 0), stop=False)
                wh8 = load_w8(WFh8d, kp, "wfh8")
                wl8 = load_w8(WFl8d, kp, "wfl8")
                for d in range(8):
                    sl = slice(128 * d, 128 * (d + 1))
                    nc.tensor.matmul(psums[d], wh8[:, :, sl], xl8,
                                     start=False, stop=False, perf_mode=DR)
                    nc.tensor.matmul(psums[d], wl8[:, :, sl], xh8,
                                     start=False, stop=(kp == 11), perf_mode=DR)
            for d in range(8):
                nc.scalar.activation(h1[:, d, :], psums[d], AF.Relu,
                                     bias=b1_sb[:, d:d + 1], scale=s1_sb[:, d:d + 1])

            # ---- W2 -> h2: tf32 hi-pass + fp8 DoubleRow lo-pass ----
            h2 = pt([128, _chunks(H), CS], F32, tag="h2", bufs=1)
            psums = [ps.tile([128, 512], F32, tag="mm", name="p2") for _ in range(8)]
            for kp in range(4):
                al8 = pt([128, 2, CS], F8, tag="al8", bufs=2)
                for kk in range(2):
                    k = 2 * kp + kk
                    ah, al = split_act(h1[:, k, :])
                    nc.scalar.activation(al8[:, kk, :], al.bitcast(F32), AF.Copy,
                                         scale=16.0)
                    wh = load_wh(W2hi, 128 * k, 0, 1024)
                    for d in range(8):
                        nc.tensor.matmul(psums[d], wh[:, 128 * d:128 * (d + 1)],
                                         ah, start=(k == 0), stop=False)
                w2h8 = load_w8(W2h8d, kp, "w2h8")
                for d in range(8):
                    sl = slice(128 * d, 128 * (d + 1))
                    nc.tensor.matmul(psums[d], w2h8[:, :, sl], al8,
                                     start=False, stop=(kp == 3), perf_mode=DR)
            for d in range(8):
                nc.scalar.activation(h2[:, d, :], psums[d], AF.Relu,
                                     bias=b2_sb[:, d:d + 1], scale=s2_sb[:, d:d + 1])

            # ---- W3, single pass (feeds LN variance only); squares -> sqacc ----
            ahh = pt([128, _chunks(H), CS], F32R, tag="ahh", bufs=1)
            for k in range(_chunks(H)):
                nc.scalar.copy(ahh[:, k, :], h2[:, k, :])
            sqacc = pt([128, CS], F32, tag="sqa", bufs=2)
            for dg in range(2):
                psums = [ps.tile([128, 512], F32, tag="mm", name="p3")
                         for _ in range(6)]
                for k in range(_chunks(H)):
                    wh = load_wh(W3hi, 128 * k, 768 * dg, 768)
                    for d in range(6):
                        nc.tensor.matmul(psums[d], wh[:, 128 * d:128 * (d + 1)],
                                         ahh[:, k, :],
                                         start=(k == 0), stop=(k == _chunks(H) - 1))
                for d in range(6):
                    dd = 6 * dg + d
                    if dd == 0:
                        nc.scalar.activation(sqacc, psums[d], AF.Square,
                                             bias=b3_sb[:, dd:dd + 1], scale=1.0)
                    else:
                        sqt = pt([128, CS], F32, tag="sqt", bufs=2)
                        nc.scalar.activation(sqt, psums[d], AF.Square,
                                             bias=b3_sb[:, dd:dd + 1], scale=1.0)
                        nc.vector.tensor_add(sqacc, sqacc, sqt)

            # ---- tail: z entirely from h2 (exact W3 fold) per 128-chunk ----
            for c in range(NCH):
                cs_ = slice(128 * c, 128 * (c + 1))
                pz = ps.tile([128, 49], F32, tag="mm", name="pz")
                pq = ps.tile([128, 1], F32, tag="mm", name="pq")
                for k in range(_chunks(H)):
                    nc.tensor.matmul(pz, h2[:, k, cs_], wzh_sb[:, k, :],
                                     start=(k == 0), stop=False)
                # + [b3 @ wz | sum(b3)] broadcast row
                nc.tensor.matmul(pz, onesr_sb, zb_sb, start=False, stop=True)
                nc.tensor.matmul(pq, sqacc[:, cs_], ones_sb,
                                 start=True, stop=True)

                # per-token LN stats (token-major [128,1])
                mu = pt([128, 1], F32, tag="mu", bufs=2)
                nc.vector.tensor_scalar(out=mu, in0=pz[:, 48:49],
                                        scalar1=1.0 / D3, scalar2=None, op0=ALU.mult)
                et2 = pt([128, 1], F32, tag="et2", bufs=2)
                nc.vector.tensor_scalar(out=et2, in0=pq, scalar1=1.0 / D3,
                                        scalar2=EPS, op0=ALU.mult, op1=ALU.add)
                msq = pt([128, 1], F32, tag="msq", bufs=2)
                nc.scalar.activation(msq, mu, AF.Square)
                veps = pt([128, 1], F32, tag="veps", bufs=2)
                nc.vector.scalar_tensor_tensor(out=veps, in0=msq, scalar=-1.0,
                                               in1=et2, op0=ALU.mult, op1=ALU.add)
                sdev = pt([128, 1], F32, tag="sdev", bufs=2)
                nc.scalar.activation(sdev, veps, AF.Sqrt)
                r_t = pt([128, 1], F32, tag="r_t", bufs=2)
                nc.vector.reciprocal(r_t, sdev)
                mr_t = pt([128, 1], F32, tag="mr_t", bufs=2)
                nc.vector.tensor_mul(mr_t, mu, r_t)

                # z = r*Z - (mu*r)*c + bias   on [128,48]
                z = pt([128, 48], F32, tag="z", bufs=2)
                nc.vector.tensor_scalar(out=z, in0=pz[:, 0:48], scalar1=r_t,
                                        scalar2=None, op0=ALU.mult)
                nc.vector.scalar_tensor_tensor(out=z, in0=cneg_bc, scalar=mr_t,
                                               in1=z, op0=ALU.mult, op1=ALU.add)
                nc.vector.tensor_add(z, z, bz_bc)

                # per part: top-2 softmax gates, then sum_e w_e * S_e
                ctbs = []
                for j in range(3):
                    lg = z[:, 16 * j:16 * j + 8]
                    Sv = z[:, 16 * j + 8:16 * j + 16]
                    max1 = pt([128, 1], F32, tag="max1", bufs=2)
                    nc.vector.reduce_max(max1, lg, axis=AX.X)
                    is1 = pt([128, 8], F32, tag="is1", bufs=2)
                    nc.vector.tensor_scalar(out=is1, in0=lg, scalar1=max1,
                                            scalar2=None, op0=ALU.is_equal)
                    l2 = pt([128, 8], F32, tag="l2", bufs=2)
                    nc.vector.scalar_tensor_tensor(out=l2, in0=is1, scalar=NEG_BIG,
                                                   in1=lg, op0=ALU.mult, op1=ALU.add)
                    max2 = pt([128, 1], F32, tag="max2", bufs=2)
                    nc.vector.reduce_max(max2, l2, axis=AX.X)
                    dlt = pt([128, 1], F32, tag="dlt", bufs=2)
                    nc.vector.tensor_sub(dlt, max1, max2)
                    s1 = pt([128, 1], F32, tag="s1", bufs=2)
                    nc.scalar.activation(s1, dlt, AF.Sigmoid)
                    s2 = pt([128, 1], F32, tag="s2", bufs=2)
                    nc.scalar.activation(s2, dlt, AF.Sigmoid, scale=-1.0)
                    is2 = pt([128, 8], F32, tag="is2", bufs=2)
                    nc.vector.tensor_scalar(out=is2, in0=l2, scalar1=max2,
                                            scalar2=None, op0=ALU.is_equal)
                    w_sb = pt([128, 8], F32, tag="w_sb", bufs=2)
                    nc.vector.tensor_scalar(out=w_sb, in0=is1, scalar1=s1,
                                            scalar2=None, op0=ALU.mult)
                    nc.vector.scalar_tensor_tensor(out=w_sb, in0=is2, scalar=s2,
                                                   in1=w_sb, op0=ALU.mult,
                                                   op1=ALU.add)
                    wS = pt([128, 8], F32, tag="wS", bufs=2)
                    ctb = pt([128, 1], F32, tag="ctb", bufs=3)
                    nc.vector.scalar_tensor_tensor(out=wS, in0=Sv, scalar=1.0,
                                                   in1=w_sb, op0=ALU.bypass,
                                                   op1=ALU.mult, accum_out=ctb)
                    ctbs.append(ctb)

                col = NCH * t + c
                c01 = pt([128, 1], F32, tag="c01", bufs=2)
                nc.vector.tensor_add(c01, ctbs[0], ctbs[1])
                nc.vector.tensor_add(out128[:, col:col + 1], c01, ctbs[2])

        nc.sync.dma_start(out=out_d[:, :], in_=out128)

    nc.compile()
    return nc


def _pack_vec(v, nch):
    return np.ascontiguousarray(np.asarray(v, np.float32).reshape(nch, 128).T)


def _tf32_split(w):
    """Split fp32 matrix into tf32-representable hi + lo (RNE at 11
    mantissa bits, matching the PE's fp32r rounding)."""
    w = np.ascontiguousarray(w, np.float32)

    def rnd(x):
        u = x.view(np.uint32)
        keep = ((u + 0x800 + ((u >> 12) & 1)) & 0xFFFFF000).astype(np.uint32)
        return keep.view(np.float32)

    hi = rnd(w)
    lo = rnd((w.astype(np.float64) - hi.astype(np.float64)).astype(np.float32))
    return hi, lo


def prepare_maps(inputs):
    """Host-side sharding + weight folding. Returns per-core input maps
    plus the global output constant c0."""
    f32, f64 = np.float32, np.float64
    k64 = 1.0 / np.sqrt(f64(1.0) + f64(EPS))
    g1 = np.asarray(inputs["g1"], f64)
    g2 = np.asarray(inputs["g2"], f64)

    # ---- fold proj into W1: WF_i = Wp_i @ W1_i ; b1' = sum_i bp_i@W1_i + b1
    W1 = np.asarray(inputs["W1"], f64)
    WF = np.concatenate(
        [np.asarray(inputs[f"Wp{i+1}"], f64) @ W1[D * i:D * (i + 1), :]
         for i in range(3)], axis=0)                        # [3*DIN, H]
    b1p = (np.concatenate([np.asarray(inputs[f"bp{i+1}"], f64)
                           for i in range(3)]) @ W1
           + np.asarray(inputs["b1"], f64))

    # ---- output-tail fold: out = concat(o) @ wfr + c0
    scf = np.asarray(inputs["bng"], f64) * k64
    wfr = np.asarray(inputs["Wf"], f64) @ (scf * np.asarray(inputs["Wr"], f64)[:, 0])
    c0 = ((np.asarray(inputs["bf"], f64) * scf + np.asarray(inputs["bnb"], f64))
          @ np.asarray(inputs["Wr"], f64)[:, 0] + f64(inputs["br"][0]))

    # ---- LN fold into router / expert-scalar weights
    lng = np.asarray(inputs["lng"], f64)
    lnb = np.asarray(inputs["lnb"], f64)
    Wg = np.asarray(inputs["Wg"], f64)
    bg = np.asarray(inputs["bg"], f64)
    We = np.asarray(inputs["We"], f64)
    bexp = np.asarray(inputs["bexp"], f64)
    wzfull = np.zeros((D3, 49), f64)
    cneg = np.zeros(48, f64)
    bz = np.zeros(48, f64)
    for j in range(3):
        sl = slice(D * j, D * (j + 1))
        lngj, lnbj, wfrj = lng[sl], lnb[sl], wfr[sl]
        Vj = (We @ wfrj).T                                  # [D, E]
        wzfull[sl, 16 * j:16 * j + 8] = lngj[:, None] * Wg
        wzfull[sl, 16 * j + 8:16 * j + 16] = lngj[:, None] * Vj
        cneg[16 * j:16 * j + 8] = -(lngj @ Wg)
        cneg[16 * j + 8:16 * j + 16] = -(lngj @ Vj)
        bz[16 * j:16 * j + 8] = bg + lnbj @ Wg
        bz[16 * j + 8:16 * j + 16] = bexp @ wfrj + lnbj @ Vj
    wzfull[:, 48] = 1.0

    WFhi, _ = _tf32_split(WF.astype(f32))
    W2hi, _ = _tf32_split(inputs["W2"])
    W3hi, _ = _tf32_split(inputs["W3"])
    WFlo64 = WF - WFhi.astype(f64)                      # exact lo residual
    E5 = ml_dtypes.float8_e5m2

    def _pack_pairs(arr):                               # [K,N] -> [128,K/256,2,N]
        K, N = arr.shape
        return np.ascontiguousarray(
            arr.reshape(K // 256, 2, 128, N).transpose(2, 0, 1, 3))
    # z comes entirely from h2:  Z = h2 @ (W3 @ wz) + b3 @ wz  (exact W3),
    # col 48 is the feature-sum for mu:  sum(t3) = h2 @ W3.sum(1) + sum(b3)
    W3f = np.asarray(inputs["W3"], f64)
    b3f = np.asarray(inputs["b3"], f64)
    wzh = np.zeros((H, 49), f64)
    wzh[:, 0:48] = W3f @ wzfull[:, 0:48]
    wzh[:, 48] = W3f.sum(1)
    zb = np.zeros(49, f64)
    zb[0:48] = b3f @ wzfull[:, 0:48]
    zb[48] = b3f.sum()
    consts = {
        "WFhi": WFhi,
        "W2hi": W2hi,
        "W3hi": W3hi,
        "WFh8": _pack_pairs((WFhi.astype(f64) / 16.0).astype(f32)).astype(E5),
        "WFl8": _pack_pairs((WFlo64 * 1024.0).astype(f32)).astype(E5),
        "W2h8": _pack_pairs((W2hi.astype(f64) / 16.0).astype(f32)).astype(E5),
        "wzh": np.ascontiguousarray(
            wzh.astype(f32).reshape(_chunks(H), 128, 49).transpose(1, 0, 2)),
        "zb": zb.astype(f32).reshape(1, 49),
        "ones_row": np.ones((1, 128), f32),
        "s1v": _pack_vec((g1 * k64).astype(f32), _chunks(H)),
        "b1v": _pack_vec((b1p * g1 * k64
                          + np.asarray(inputs["be1"], f64)).astype(f32), _chunks(H)),
        "s2v": _pack_vec((g2 * k64).astype(f32), _chunks(H)),
        "b2v": _pack_vec((np.asarray(inputs["b2"], f64) * g2 * k64
                          + np.asarray(inputs["be2"], f64)).astype(f32), _chunks(H)),
        "b3v": _pack_vec(inputs["b3"], _chunks(D3)),
        "cneg": cneg.astype(f32).reshape(1, 48),
        "bz": bz.astype(f32).reshape(1, 48),
        "ones_col": np.ones((128, 1), f32),
    }
    xts = [np.ascontiguousarray(np.asarray(inputs[f"x{i+1}"], f32).T)
           for i in range(3)]
    in_maps = []
    for c in range(N_CORES):
        m = dict(consts)
        sl = slice(c * TOK_CORE, (c + 1) * TOK_CORE)
        for i in range(3):
            m[f"x{i+1}t"] = np.ascontiguousarray(xts[i][:, sl])
        in_maps.append(m)
    return in_maps, c0


def run(inputs, trace=False, n_tok=TOK_CORE):
    key = n_tok
    if key not in _PROGRAM_CACHE:
        _PROGRAM_CACHE[key] = build_program(n_tok=n_tok)
    nc = _PROGRAM_CACHE[key]
    in_maps, c0 = prepare_maps(inputs)
    res = run_bass_kernel_spmd(nc, in_maps, list(range(N_CORES)), trace=trace)
    rows = []
    for c in range(N_CORES):
        arr = res.results[c]["out"]            # [128, NCOL]; token = col*128 + row
        rows.append(np.ascontiguousarray(arr.T).reshape(-1))
    out = (np.concatenate(rows).astype(np.float64) + c0).astype(np.float32)
    return out.reshape(B, 1), res


def kernel(**inputs):
    out, _ = run(inputs, trace=False)
    return out


# revision 34
# speedup vs baseline: 2.5968x; 1.0591x over previous
"""Trainium2 Bass kernel for nn_CombinedMLPMoEModel (moe_routing).

Strategy (8 NeuronCores, pure data parallel on the batch):
 - Host: shard batch 16384 -> 8 x 2048 tokens, pre-transpose x1/x2/x3 to
   feature-major [Din, tok]; replicate weights.
 - The final output is a scalar per token: concat(o1,o2,o3) @ Wf -> bn
   -> @ Wr.  That tail is linear, so each MoE expert's contribution
   collapses to a per-token SCALAR:
       o_j . wfr_j = sum_k g_k (m_j . (W_e @ wfr_j) + b_e . wfr_j)
   with wfr = Wf @ (bn_scale * Wr).  The dense [512x512] expert matmuls
   disappear; per part we need one [512 -> 8] matmul (like the router),
   computed in exact fp32.
 - LayerNorm is linear per token, so it folds into those matmuls:
       logits = r*(t3 @ (lng.Wg)) - (mu*r)*c_g + bias_g     (same for S)
   i.e. no normalized tensor is ever materialized; the per-token affine
   (r, mu*r) is applied on [128,48] tiles.
 - The proj layer folds into W1 on the host (WF_i = Wp_i @ W1_i; same
   FLOPs, one less pipeline stage).
 - MLP chain precision: fp32r (tf32-like) 3-term split
       W @ x ~= Whi@xhi + Whi@xlo + Wlo@xhi
   with Whi/Wlo PRE-SPLIT ON THE HOST (doubles weight DMA, removes the
   on-chip weight-split engine work).  Routing (top-2 of 8) then matches
   the fp32 reference exactly (0 flips on the reference inputs; total
   rel err ~1e-5).
 - LN stats token-major: sum(t3) rides as a free ones-column (col 48) of
   the z-matmul; sum(t3^2) via a squares tensor against a ones vector.
"""

import numpy as np
import ml_dtypes
from contextlib import ExitStack

import concourse.bass as bass
from concourse import bacc
import concourse.mybir as mybir
import concourse.tile as tile
from concourse.bass_utils import run_bass_kernel_spmd

F32 = mybir.dt.float32
F32R = mybir.dt.float32r
F8 = mybir.dt.float8e5
DR = mybir.MatmulPerfMode.DoubleRow
AF = mybir.ActivationFunctionType
ALU = mybir.AluOpType
AX = mybir.AxisListType

N_CORES = 8
B = 16384
DIN = 1024
D = 512
H = 1024
D3 = 3 * D          # 1536
E = 8
TOK_CORE = B // N_CORES   # 2048
MT = 512                  # megatile tokens
EPS = 1e-5
NEG_BIG = -1.0e30

_PROGRAM_CACHE = {}


def _chunks(n):
    return n // 128


def build_program(n_tok=TOK_CORE, mt=MT):
    """Build the per-core Bass program (SPMD across the 8 cores)."""
    nc = bacc.Bacc(None, target_bir_lowering=False)
    T = n_tok // mt
    CS = mt
    NCH = _chunks(mt)          # 128-token chunks per megatile
    NCOL = T * NCH             # output columns per core

    # ---------------- DRAM I/O ----------------
    xs = [nc.dram_tensor(f"x{i+1}t", [DIN, n_tok], F32, kind="ExternalInput")
          for i in range(3)]
    WFhi = nc.dram_tensor("WFhi", [3 * DIN, H], F32R, kind="ExternalInput")
    W2hi = nc.dram_tensor("W2hi", [H, H], F32R, kind="ExternalInput")
    W3hi = nc.dram_tensor("W3hi", [H, D3], F32R, kind="ExternalInput")
    # fp8-e5m2 lo-pass weights, packed as [128, kpair, 2, dout]
    WFh8d = nc.dram_tensor("WFh8", [128, 12, 2, H], F8, kind="ExternalInput")
    WFl8d = nc.dram_tensor("WFl8", [128, 12, 2, H], F8, kind="ExternalInput")
    W2h8d = nc.dram_tensor("W2h8", [128, 4, 2, H], F8, kind="ExternalInput")
    s1d = nc.dram_tensor("s1v", [128, _chunks(H)], F32, kind="ExternalInput")
    b1d = nc.dram_tensor("b1v", [128, _chunks(H)], F32, kind="ExternalInput")
    s2d = nc.dram_tensor("s2v", [128, _chunks(H)], F32, kind="ExternalInput")
    b2d = nc.dram_tensor("b2v", [128, _chunks(H)], F32, kind="ExternalInput")
    b3d = nc.dram_tensor("b3v", [128, _chunks(D3)], F32, kind="ExternalInput")
    wzhd = nc.dram_tensor("wzh", [128, _chunks(H), 49], F32, kind="ExternalInput")
    zbd = nc.dram_tensor("zb", [1, 49], F32, kind="ExternalInput")
    onesrd = nc.dram_tensor("ones_row", [1, 128], F32, kind="ExternalInput")
    cnegd = nc.dram_tensor("cneg", [1, 48], F32, kind="ExternalInput")
    bzd = nc.dram_tensor("bz", [1, 48], F32, kind="ExternalInput")
    onesd = nc.dram_tensor("ones_col", [128, 1], F32, kind="ExternalInput")
    out_d = nc.dram_tensor("out", [128, NCOL], F32, kind="ExternalOutput")

    with tile.TileContext(nc) as tc, ExitStack() as ctx:
        cp = ctx.enter_context(tc.tile_pool(name="consts", bufs=1))
        sp = ctx.enter_context(tc.tile_pool(name="work", bufs=1))
        ps = ctx.enter_context(tc.tile_pool(name="psum", bufs=8, space="PSUM"))

        def pt(shape, dtype, tag, bufs=None):
            return sp.tile(shape, dtype, tag=tag, bufs=bufs, name=tag)

        # ---------------- resident constants ----------------
        # consts ride the gpsimd DMA queue so the first megatile's x/weight
        # DMAs on the sync queue are not delayed behind them
        wzh_sb = cp.tile([128, _chunks(H), 49], F32, name="wzh_sb")
        nc.gpsimd.dma_start(out=wzh_sb, in_=wzhd[:, :, :])
        zb_sb = cp.tile([1, 49], F32, name="zb_sb")
        nc.gpsimd.dma_start(out=zb_sb, in_=zbd[:, :])
        onesr_sb = cp.tile([1, 128], F32, name="onesr_sb")
        nc.gpsimd.dma_start(out=onesr_sb, in_=onesrd[:, :])
        ones_sb = cp.tile([128, 1], F32, name="ones_sb")
        nc.gpsimd.dma_start(out=ones_sb, in_=onesd[:, :])
        cneg_bc = cp.tile([128, 48], F32, name="cneg_bc")
        nc.gpsimd.dma_start(
            out=cneg_bc,
            in_=bass.AP(tensor=cnegd[:, :].tensor, offset=0, ap=[[0, 128], [1, 48]]),
        )
        bz_bc = cp.tile([128, 48], F32, name="bz_bc")
        nc.gpsimd.dma_start(
            out=bz_bc,
            in_=bass.AP(tensor=bzd[:, :].tensor, offset=0, ap=[[0, 128], [1, 48]]),
        )

        def ldvec(dram, nch, name):
            t = cp.tile([128, nch], F32, name=name)
            nc.gpsimd.dma_start(out=t, in_=dram[:, :])
            return t

        s1_sb = ldvec(s1d, _chunks(H), "s1_sb")
        b1_sb = ldvec(b1d, _chunks(H), "b1_sb")
        s2_sb = ldvec(s2d, _chunks(H), "s2_sb")
        b2_sb = ldvec(b2d, _chunks(H), "b2_sb")
        b3_sb = ldvec(b3d, _chunks(D3), "b3_sb")

        out128 = cp.tile([128, NCOL], F32, name="out128")

        def split_act(src_ap):
            """tf32 hi/lo split of one [128, CS] fp32 activation chunk."""
            hi = pt([128, CS], F32R, tag="aph", bufs=2)
            nc.scalar.copy(hi, src_ap)
            lo = pt([128, CS], F32R, tag="apl", bufs=2)
            nc.vector.scalar_tensor_tensor(out=lo, in0=src_ap, scalar=-1.0,
                                           in1=hi.bitcast(F32), op0=ALU.bypass,
                                           op1=ALU.subtract)
            return hi, lo

        def load_wh(hi_dram, r0, dgs, dgw, tag):
            """DMA pre-split tf32 hi weight chunk (rows [r0, r0+128),
            dout cols [dgs, dgs+dgw)). Per-layer tag so next layer's
            prefetch is not serialized on this layer's buffers."""
            wh = pt([128, 1024], F32R, tag=tag, bufs=3)[:, :dgw]
            nc.sync.dma_start(out=wh, in_=hi_dram[r0:r0 + 128, dgs:dgs + dgw])
            return wh

        def load_w8(dram8, kp, tag):
            w8 = pt([128, 2, H], F8, tag=tag, bufs=3)
            nc.sync.dma_start(out=w8, in_=dram8[:, kp, :, :])
            return w8

        # ---------------- megatile loop ----------------
        for t in range(T):
            ts = slice(t * CS, (t + 1) * CS)

            # ---- W1F: h1 = relu(bn(sum_i x_i @ WF_i + b1')) ----
            # pass 1: tf32 Whi@xhi per k-chunk; passes 2+3: fp8-e5m2
            # DoubleRow over k-pairs (Whi@xlo and Wlo@xhi, scale-neutral).
            h1 = pt([128, _chunks(H), CS], F32, tag="h1", bufs=1)
            psums = [ps.tile([128, 512], F32, tag="mm", name="p1") for _ in range(8)]
            for kp in range(12):
                xh8 = pt([128, 2, CS], F8, tag="xh8", bufs=2)
                xl8 = pt([128, 2, CS], F8, tag="xl8", bufs=2)
                for kk in range(2):
                    kc = 2 * kp + kk
                    i, k = kc // _chunks(DIN), kc % _chunks(DIN)
                    xk = pt([128, CS], F32, tag="xk", bufs=3)
                    nc.sync.dma_start(out=xk, in_=xs[i][128 * k:128 * (k + 1), ts])
                    xh, xl = split_act(xk)
                    nc.scalar.activation(xh8[:, kk, :], xh.bitcast(F32), AF.Copy,
                                         scale=1.0 / 1024.0)
                    nc.scalar.activation(xl8[:, kk, :], xl.bitcast(F32), AF.Copy,
                                         scale=16.0)
                    wh = load_wh(WFhi, 128 * kc, 0, 1024, "whf")
                    for d in range(8):
                        nc.tensor.matmul(psums[d], wh[:, 128 * d:128 * (d + 1)],
                                         xh, start=(kc == 0), stop=False)
                wh8 = load_w8(WFh8d, kp, "wfh8")
                wl8 = load_w8(WFl8d, kp, "wfl8")
                for d in range(8):
                    sl = slice(128 * d, 128 * (d + 1))
                    nc.tensor.matmul(psums[d], wh8[:, :, sl], xl8,
                                     start=False, stop=False, perf_mode=DR)
                    nc.tensor.matmul(psums[d], wl8[:, :, sl], xh8,
                                     start=False, stop=(kp == 11), perf_mode=DR)
            for d in range(8):
                nc.scalar.activation(h1[:, d, :], psums[d], AF.Relu,
                                     bias=b1_sb[:, d:d + 1], scale=s1_sb[:, d:d + 1])

            # ---- W2 -> h2: tf32 hi-pass + fp8 DoubleRow lo-pass ----
            h2 = pt([128, _chunks(H), CS], F32, tag="h2", bufs=1)
            psums = [ps.tile([128, 512], F32, tag="mm", name="p2") for _ in range(8)]
            for kp in range(4):
                al8 = pt([128, 2, CS], F8, tag="al8", bufs=2)
                for kk in range(2):
                    k = 2 * kp + kk
                    ah, al = split_act(h1[:, k, :])
                    nc.scalar.activation(al8[:, kk, :], al.bitcast(F32), AF.Copy,
                                         scale=16.0)
                    wh = load_wh(W2hi, 128 * k, 0, 1024, "wh2")
                    for d in range(8):
                        nc.tensor.matmul(psums[d], wh[:, 128 * d:128 * (d + 1)],
                                         ah, start=(k == 0), stop=False)
                w2h8 = load_w8(W2h8d, kp, "w2h8")
                for d in range(8):
                    sl = slice(128 * d, 128 * (d + 1))
                    nc.tensor.matmul(psums[d], w2h8[:, :, sl], al8,
                                     start=False, stop=(kp == 3), perf_mode=DR)
            for d in range(8):
                nc.scalar.activation(h2[:, d, :], psums[d], AF.Relu,
                                     bias=b2_sb[:, d:d + 1], scale=s2_sb[:, d:d + 1])

            # ---- W3, single pass (feeds LN variance only); squares -> sqacc ----
            ahh = pt([128, _chunks(H), CS], F32R, tag="ahh", bufs=1)
            for k in range(_chunks(H)):
                nc.scalar.copy(ahh[:, k, :], h2[:, k, :])
            sqacc = pt([128, CS], F32, tag="sqa", bufs=2)
            for dg in range(2):
                psums = [ps.tile([128, 512], F32, tag="mm", name="p3")
                         for _ in range(6)]
                for k in range(_chunks(H)):
                    wh = load_wh(W3hi, 128 * k, 768 * dg, 768, "wh3")
                    for d in range(6):
                        nc.tensor.matmul(psums[d], wh[:, 128 * d:128 * (d + 1)],
                                         ahh[:, k, :],
                                         start=(k == 0), stop=(k == _chunks(H) - 1))
                for d in range(6):
                    dd = 6 * dg + d
                    if dd == 0:
                        nc.scalar.activation(sqacc, psums[d], AF.Square,
                                             bias=b3_sb[:, dd:dd + 1], scale=1.0)
                    else:
                        sqt = pt([128, CS], F32, tag="sqt", bufs=2)
                        nc.scalar.activation(sqt, psums[d], AF.Square,
                                             bias=b3_sb[:, dd:dd + 1], scale=1.0)
                        nc.vector.tensor_add(sqacc, sqacc, sqt)

            # ---- tail: z entirely from h2 (exact W3 fold) per 128-chunk ----
            # pq shares pz's bank (col 49): keeps psum allocations/megatile at
            # a multiple of 8 so next megatile's W1F psums reuse long-freed
            # banks instead of waiting on the tail. pz's first matmul
            # (start=True) cleared the whole bank, so the col-49 write with
            # start=False lands fresh.
            for c in range(NCH):
                cs_ = slice(128 * c, 128 * (c + 1))
                pz = ps.tile([128, 50], F32, tag="mm", name="pz")
                for k in range(_chunks(H)):
                    nc.tensor.matmul(pz[:, 0:49], h2[:, k, cs_], wzh_sb[:, k, :],
                                     start=(k == 0), stop=False)
                # + [b3 @ wz | sum(b3)] broadcast row
                nc.tensor.matmul(pz[:, 0:49], onesr_sb, zb_sb,
                                 start=False, stop=True)
                nc.tensor.matmul(pz[:, 49:50], sqacc[:, cs_], ones_sb,
                                 start=False, stop=True, skip_group_check=True)

                # per-token LN stats (token-major [128,1])
                mu = pt([128, 1], F32, tag="mu", bufs=2)
                nc.vector.tensor_scalar(out=mu, in0=pz[:, 48:49],
                                        scalar1=1.0 / D3, scalar2=None, op0=ALU.mult)
                et2 = pt([128, 1], F32, tag="et2", bufs=2)
                nc.vector.tensor_scalar(out=et2, in0=pz[:, 49:50], scalar1=1.0 / D3,
                                        scalar2=EPS, op0=ALU.mult, op1=ALU.add)
                msq = pt([128, 1], F32, tag="msq", bufs=2)
                nc.scalar.activation(msq, mu, AF.Square)
                veps = pt([128, 1], F32, tag="veps", bufs=2)
                nc.vector.scalar_tensor_tensor(out=veps, in0=msq, scalar=-1.0,
                                               in1=et2, op0=ALU.mult, op1=ALU.add)
                sdev = pt([128, 1], F32, tag="sdev", bufs=2)
                nc.scalar.activation(sdev, veps, AF.Sqrt)
                r_t = pt([128, 1], F32, tag="r_t", bufs=2)
                nc.vector.reciprocal(r_t, sdev)
                mr_t = pt([128, 1], F32, tag="mr_t", bufs=2)
                nc.vector.tensor_mul(mr_t, mu, r_t)

                # z = r*Z - (mu*r)*c + bias   on [128,48]
                z = pt([128, 48], F32, tag="z", bufs=2)
                nc.vector.tensor_scalar(out=z, in0=pz[:, 0:48], scalar1=r_t,
                                        scalar2=None, op0=ALU.mult)
                nc.vector.scalar_tensor_tensor(out=z, in0=cneg_bc, scalar=mr_t,
                                               in1=z, op0=ALU.mult, op1=ALU.add)
                nc.vector.tensor_add(z, z, bz_bc)

                # per part: top-2 softmax gates, then sum_e w_e * S_e
                ctbs = []
                for j in range(3):
                    lg = z[:, 16 * j:16 * j + 8]
                    Sv = z[:, 16 * j + 8:16 * j + 16]
                    max1 = pt([128, 1], F32, tag="max1", bufs=2)
                    nc.vector.reduce_max(max1, lg, axis=AX.X)
                    is1 = pt([128, 8], F32, tag="is1", bufs=2)
                    nc.vector.tensor_scalar(out=is1, in0=lg, scalar1=max1,
                                            scalar2=None, op0=ALU.is_equal)
                    l2 = pt([128, 8], F32, tag="l2", bufs=2)
                    nc.vector.scalar_tensor_tensor(out=l2, in0=is1, scalar=NEG_BIG,
                                                   in1=lg, op0=ALU.mult, op1=ALU.add)
                    max2 = pt([128, 1], F32, tag="max2", bufs=2)
                    nc.vector.reduce_max(max2, l2, axis=AX.X)
                    dlt = pt([128, 1], F32, tag="dlt", bufs=2)
                    nc.vector.tensor_sub(dlt, max1, max2)
                    s1 = pt([128, 1], F32, tag="s1", bufs=2)
                    nc.scalar.activation(s1, dlt, AF.Sigmoid)
                    s2 = pt([128, 1], F32, tag="s2", bufs=2)
                    nc.scalar.activation(s2, dlt, AF.Sigmoid, scale=-1.0)
                    is2 = pt([128, 8], F32, tag="is2", bufs=2)
                    nc.vector.tensor_scalar(out=is2, in0=l2, scalar1=max2,
                                            scalar2=None, op0=ALU.is_equal)
                    w_sb = pt([128, 8], F32, tag="w_sb", bufs=2)
                    nc.vector.tensor_scalar(out=w_sb, in0=is1, scalar1=s1,
                                            scalar2=None, op0=ALU.mult)
                    nc.vector.scalar_tensor_tensor(out=w_sb, in0=is2, scalar=s2,
                                                   in1=w_sb, op0=ALU.mult,
                                                   op1=ALU.add)
                    wS = pt([128, 8], F32, tag="wS", bufs=2)
                    ctb = pt([128, 1], F32, tag="ctb", bufs=3)
                    nc.vector.scalar_tensor_tensor(out=wS, in0=Sv, scalar=1.0,
                                                   in1=w_sb, op0=ALU.bypass,
                                                   op1=ALU.mult, accum_out=ctb)
                    ctbs.append(ctb)

                col = NCH * t + c
                c01 = pt([128, 1], F32, tag="c01", bufs=2)
                nc.vector.tensor_add(c01, ctbs[0], ctbs[1])
                nc.vector.tensor_add(out128[:, col:col + 1], c01, ctbs[2])

        nc.sync.dma_start(out=out_d[:, :], in_=out128)

    nc.compile()
    return nc


def _pack_vec(v, nch):
    return np.ascontiguousarray(np.asarray(v, np.float32).reshape(nch, 128).T)


def _tf32_split(w):
    """Split fp32 matrix into tf32-representable hi + lo (RNE at 11
    mantissa bits, matching the PE's fp32r rounding)."""
    w = np.ascontiguousarray(w, np.float32)

    def rnd(x):
        u = x.view(np.uint32)
        keep = ((u + 0x800 + ((u >> 12) & 1)) & 0xFFFFF000).astype(np.uint32)
        return keep.view(np.float32)

    hi = rnd(w)
    lo = rnd((w.astype(np.float64) - hi.astype(np.float64)).astype(np.float32))
    return hi, lo


def prepare_maps(inputs):
    """Host-side sharding + weight folding. Returns per-core input maps
    plus the global output constant c0."""
    f32, f64 = np.float32, np.float64
    k64 = 1.0 / np.sqrt(f64(1.0) + f64(EPS))
    g1 = np.asarray(inputs["g1"], f64)
    g2 = np.asarray(inputs["g2"], f64)

    # ---- fold proj into W1: WF_i = Wp_i @ W1_i ; b1' = sum_i bp_i@W1_i + b1
    W1 = np.asarray(inputs["W1"], f64)
    WF = np.concatenate(
        [np.asarray(inputs[f"Wp{i+1}"], f64) @ W1[D * i:D * (i + 1), :]
         for i in range(3)], axis=0)                        # [3*DIN, H]
    b1p = (np.concatenate([np.asarray(inputs[f"bp{i+1}"], f64)
                           for i in range(3)]) @ W1
           + np.asarray(inputs["b1"], f64))

    # ---- output-tail fold: out = concat(o) @ wfr + c0
    scf = np.asarray(inputs["bng"], f64) * k64
    wfr = np.asarray(inputs["Wf"], f64) @ (scf * np.asarray(inputs["Wr"], f64)[:, 0])
    c0 = ((np.asarray(inputs["bf"], f64) * scf + np.asarray(inputs["bnb"], f64))
          @ np.asarray(inputs["Wr"], f64)[:, 0] + f64(inputs["br"][0]))

    # ---- LN fold into router / expert-scalar weights
    lng = np.asarray(inputs["lng"], f64)
    lnb = np.asarray(inputs["lnb"], f64)
    Wg = np.asarray(inputs["Wg"], f64)
    bg = np.asarray(inputs["bg"], f64)
    We = np.asarray(inputs["We"], f64)
    bexp = np.asarray(inputs["bexp"], f64)
    wzfull = np.zeros((D3, 49), f64)
    cneg = np.zeros(48, f64)
    bz = np.zeros(48, f64)
    for j in range(3):
        sl = slice(D * j, D * (j + 1))
        lngj, lnbj, wfrj = lng[sl], lnb[sl], wfr[sl]
        Vj = (We @ wfrj).T                                  # [D, E]
        wzfull[sl, 16 * j:16 * j + 8] = lngj[:, None] * Wg
        wzfull[sl, 16 * j + 8:16 * j + 16] = lngj[:, None] * Vj
        cneg[16 * j:16 * j + 8] = -(lngj @ Wg)
        cneg[16 * j + 8:16 * j + 16] = -(lngj @ Vj)
        bz[16 * j:16 * j + 8] = bg + lnbj @ Wg
        bz[16 * j + 8:16 * j + 16] = bexp @ wfrj + lnbj @ Vj
    wzfull[:, 48] = 1.0

    WFhi, _ = _tf32_split(WF.astype(f32))
    W2hi, _ = _tf32_split(inputs["W2"])
    W3hi, _ = _tf32_split(inputs["W3"])
    WFlo64 = WF - WFhi.astype(f64)                      # exact lo residual
    E5 = ml_dtypes.float8_e5m2

    def _pack_pairs(arr):                               # [K,N] -> [128,K/256,2,N]
        K, N = arr.shape
        return np.ascontiguousarray(
            arr.reshape(K // 256, 2, 128, N).transpose(2, 0, 1, 3))
    # z comes entirely from h2:  Z = h2 @ (W3 @ wz) + b3 @ wz  (exact W3),
    # col 48 is the feature-sum for mu:  sum(t3) = h2 @ W3.sum(1) + sum(b3)
    W3f = np.asarray(inputs["W3"], f64)
    b3f = np.asarray(inputs["b3"], f64)
    wzh = np.zeros((H, 49), f64)
    wzh[:, 0:48] = W3f @ wzfull[:, 0:48]
    wzh[:, 48] = W3f.sum(1)
    zb = np.zeros(49, f64)
    zb[0:48] = b3f @ wzfull[:, 0:48]
    zb[48] = b3f.sum()
    consts = {
        "WFhi": WFhi,
        "W2hi": W2hi,
        "W3hi": W3hi,
        "WFh8": _pack_pairs((WFhi.astype(f64) / 16.0).astype(f32)).astype(E5),
        "WFl8": _pack_pairs((WFlo64 * 1024.0).astype(f32)).astype(E5),
        "W2h8": _pack_pairs((W2hi.astype(f64) / 16.0).astype(f32)).astype(E5),
        "wzh": np.ascontiguousarray(
            wzh.astype(f32).reshape(_chunks(H), 128, 49).transpose(1, 0, 2)),
        "zb": zb.astype(f32).reshape(1, 49),
        "ones_row": np.ones((1, 128), f32),
        "s1v": _pack_vec((g1 * k64).astype(f32), _chunks(H)),
        "b1v": _pack_vec((b1p * g1 * k64
                          + np.asarray(inputs["be1"], f64)).astype(f32), _chunks(H)),
        "s2v": _pack_vec((g2 * k64).astype(f32), _chunks(H)),
        "b2v": _pack_vec((np.asarray(inputs["b2"], f64) * g2 * k64
                          + np.asarray(inputs["be2"], f64)).astype(f32), _chunks(H)),
        "b3v": _pack_vec(inputs["b3"], _chunks(D3)),
        "cneg": cneg.astype(f32).reshape(1, 48),
        "bz": bz.astype(f32).reshape(1, 48),
        "ones_col": np.ones((128, 1), f32),
    }
    xts = [np.ascontiguousarray(np.asarray(inputs[f"x{i+1}"], f32).T)
           for i in range(3)]
    in_maps = []
    for c in range(N_CORES):
        m = dict(consts)
        sl = slice(c * TOK_CORE, (c + 1) * TOK_CORE)
        for i in range(3):
            m[f"x{i+1}t"] = np.ascontiguousarray(xts[i][:, sl])
        in_maps.append(m)
    return in_maps, c0


def run(inputs, trace=False, n_tok=TOK_CORE):
    key = n_tok
    if key not in _PROGRAM_CACHE:
        _PROGRAM_CACHE[key] = build_program(n_tok=n_tok)
    nc = _PROGRAM_CACHE[key]
    in_maps, c0 = prepare_maps(inputs)
    res = run_bass_kernel_spmd(nc, in_maps, list(range(N_CORES)), trace=trace)
    rows = []
    for c in range(N_CORES):
        arr = res.results[c]["out"]            # [128, NCOL]; token = col*128 + row
        rows.append(np.ascontiguousarray(arr.T).reshape(-1))
    out = (np.concatenate(rows).astype(np.float64) + c0).astype(np.float32)
    return out.reshape(B, 1), res


def kernel(**inputs):
    out, _ = run(inputs, trace=False)
    return out
